# revision 62
# baseline (speedup 1.0000x reference)
"""AdaConv2d fused kernel for 8 TRN2 NeuronCores (pure data parallel).

Per-sample pipeline (all fused on-chip):
  1. instance-norm stats (mean/var over HW)
  2. dynamic per-(b,c) depthwise 3x3 conv with reflect padding
  3. per-(b,c) scale+bias (folded algebraically into the depthwise taps:
     y = A*(sum_t w_t * x_t) + B with A = rstd*w_pt, B = bias - mu*A*sum(w))
  4. fixed 3x3 conv (256->256) with reflect padding, as 18 accumulated
     bf16 matmuls per PSUM block

Layout: channels on partitions (2 tiles of 128), pixels on the free axis.
Padded images are 66 rows x 66 cols stored flat with a 2-element leading
margin (so every depthwise tap and every matmul rhs is a fully CONTIGUOUS
1D slice).  flat(r, c) = 2 + 66*r + c.  Rows 0/65 and cols 0/65 are the
reflect pads.  A one-element-left-shifted copy (xpb2[i] = xpb[i+1]) keeps
all odd-offset depthwise taps 4-byte aligned for the DVE bf16 2x mode.

Perf notes (measured on hw):
  - bf16 matmul N=512 paces at ~216 ns warm; 3D-AP rhs costs ~nothing.
    Slow stretches in traces are PE p-state/DVFS ramp, not AP shape.
  - DVE: TS muls ~2x (1.25us/4224), TT adds 1x (2.35us), STT and custom
    DVE ops 0.5x (4.6us) -> the TS-mul + TT-add-tree depthwise is optimal.
  - fp8 DoubleRow matmul paces 427ns for K=256 = zero gain over 2x bf16.
  - First-batch critical path: x DMA in 4 slices, ACT converts first,
    PE depthwise uses UNSCALED taps (stats fold into the PSUM drain) so
    the first matmul doesn't wait for stats.
"""

import os
from contextlib import ExitStack

import numpy as np

B_GLOBAL = 32
N_CORES = 8
NB = B_GLOBAL // N_CORES  # batches per core
C = 256
H = W = 64
WPAD = W + 2        # 66 padded row length
HPAD = H + 2        # 66 padded rows
MARG = 2            # leading margin so tap windows stay in-bounds
FLAT = MARG + HPAD * WPAD + 2   # 4360 flat elements per padded image
NPIX = H * W        # 4096
CT = C // 128       # channel tiles
OT = C // 128       # out-channel tiles
EPS = 1e-5
BLK_ROWS = 8        # output rows per PSUM block (8*64=512 fp32, 3D-AP rhs)

ROW_BLOCKS = [(r0, BLK_ROWS) for r0 in range(0, H, BLK_ROWS)]
OUT_SLICE = 16      # rows per output DMA slice (tail-latency hiding)

_CACHED = {}


def _build(nb=NB):
    import concourse.mybir as mybir
    import concourse.tile as tile
    from concourse import bacc

    f32 = mybir.dt.float32
    bf16 = mybir.dt.bfloat16
    AF = mybir.ActivationFunctionType
    ALU = mybir.AluOpType

    nc = bacc.Bacc(None, target_bir_lowering=False)

    # x and out travel as bf16 (host converts): halves both big DMA streams;
    # measured end-to-end numeric impact ~3.2e-3 rel err (vs 2e-2 budget).
    x_ext = nc.declare_dram_parameter("x", [nb, C, H, W], bf16, isOutput=False)
    wsp_ext = nc.declare_dram_parameter("wsp", [nb, CT, 128, 9], f32, isOutput=False)
    wpt_ext = nc.declare_dram_parameter("wpt", [nb, CT, 128], f32, isOutput=False)
    bis_ext = nc.declare_dram_parameter("bis", [nb, CT, 128], f32, isOutput=False)
    cw_ext = nc.declare_dram_parameter("cw", [CT, 128, 3, 3, OT, 128], bf16, isOutput=False)
    cb_ext = nc.declare_dram_parameter("cb", [OT, 128], f32, isOutput=False)
    out_ext = nc.declare_dram_parameter("out", [nb, C, H, W], bf16, isOutput=True)

    with tile.TileContext(nc) as tc, ExitStack() as ctx:
        singles = ctx.enter_context(tc.tile_pool(name="singles", bufs=1))
        xin_pool = ctx.enter_context(tc.tile_pool(name="xin", bufs=4))
        xpb_pool = ctx.enter_context(tc.tile_pool(name="xpb", bufs=2))
        xpb2_pool = ctx.enter_context(tc.tile_pool(name="xpb2", bufs=2))
        yp_pool = ctx.enter_context(tc.tile_pool(name="yp", bufs=4))
        stage_pool = ctx.enter_context(tc.tile_pool(name="stage", bufs=2))
        small_pool = ctx.enter_context(tc.tile_pool(name="small", bufs=6))
        tmp_pool = ctx.enter_context(tc.tile_pool(name="tmp", bufs=2))
        psum_pool = ctx.enter_context(tc.tile_pool(name="psum", bufs=8, space="PSUM"))

        # ---- constants / fixed weights ----
        # (emitted lazily AFTER batch 0's x DMAs so the 1.2MB weight load
        # doesn't delay the first tile's data; weights are only needed at
        # ~40us when the first big-conv matmul fires)
        cw_sb = []

        cb_sb = singles.tile([128, OT], f32, tag="cb")

        def load_cw():
            for ct in range(CT):
                t = singles.tile([128, 3, 3, OT, 128], bf16, tag=f"cw{ct}")
                nc.sync.dma_start(out=t[:], in_=cw_ext[ct])
                cw_sb.append(t)
            for ot in range(OT):
                nc.sync.dma_start(out=cb_sb[:, ot : ot + 1], in_=cb_ext[ot, :, None])
        eps_sb = singles.tile([128, 1], f32, tag="eps")
        nc.vector.memset(eps_sb[:], EPS)
        ident = singles.tile([128, 128], bf16, tag="ident")
        from concourse.masks import make_identity
        make_identity(nc, ident[:])
        # touch the Sqrt activation table once so its ~1.3us load happens
        # before the first tile's stats need it
        warm = singles.tile([128, 1], f32, tag="warm")
        nc.scalar.activation(out=warm[:], in_=eps_sb[:], func=AF.Sqrt, bias=eps_sb[:])

        yp_tiles = {}

        def grid(flat_ap):
            """(128, FLAT) flat padded buffer -> (128, 66, 66) image view."""
            return flat_ap[:, MARG : MARG + HPAD * WPAD].rearrange(
                "p (r c) -> p r c", c=WPAD)

        def fill_borders(buf):
            g = grid(buf[:])
            nc.scalar.copy(out=g[:, 1 : 1 + H, 0:1], in_=g[:, 1 : 1 + H, 2:3])
            nc.scalar.copy(out=g[:, 1 : 1 + H, 65:66], in_=g[:, 1 : 1 + H, 63:64])
            nc.scalar.copy(out=g[:, 0], in_=g[:, 2])
            nc.scalar.copy(out=g[:, HPAD - 1], in_=g[:, HPAD - 3])

        def stats_finalize(sumx, sumsq, wsp, wpt, bis, scale_taps):
            """Common stats tail: returns (a_sc, bconst, wsc-or-None)."""
            sx = small_pool.tile([128, 1], f32, tag="sx")
            nc.vector.reduce_sum(sx[:], sumx[:], axis=mybir.AxisListType.X)
            sq = small_pool.tile([128, 1], f32, tag="sq")
            nc.vector.reduce_sum(sq[:], sumsq[:], axis=mybir.AxisListType.X)
            mu = small_pool.tile([128, 1], f32, tag="mu")
            nc.vector.tensor_scalar_mul(mu[:], sx[:], 1.0 / NPIX)
            m2 = small_pool.tile([128, 1], f32, tag="m2")
            nc.vector.tensor_scalar_mul(m2[:], sq[:], 1.0 / NPIX)
            musq = small_pool.tile([128, 1], f32, tag="musq")
            nc.vector.tensor_mul(musq[:], mu[:], mu[:])
            var = small_pool.tile([128, 1], f32, tag="var")
            nc.vector.tensor_sub(var[:], m2[:], musq[:])
            std = small_pool.tile([128, 1], f32, tag="std")
            nc.scalar.activation(out=std[:], in_=var[:], func=AF.Sqrt, bias=eps_sb[:])
            rstd = small_pool.tile([128, 1], f32, tag="rstd")
            nc.vector.reciprocal(out=rstd[:], in_=std[:])
            a_sc = small_pool.tile([128, 1], f32, tag="a_sc")
            nc.vector.tensor_mul(a_sc[:], rstd[:], wpt[:])
            sw = small_pool.tile([128, 1], f32, tag="sw")
            nc.vector.reduce_sum(sw[:], wsp[:], axis=mybir.AxisListType.X)
            t1 = small_pool.tile([128, 1], f32, tag="t1")
            nc.vector.tensor_mul(t1[:], mu[:], a_sc[:])
            nc.vector.tensor_mul(t1[:], t1[:], sw[:])
            bconst = small_pool.tile([128, 1], f32, tag="bconst")
            nc.vector.tensor_sub(bconst[:], bis[:], t1[:])
            if not scale_taps:
                return a_sc, bconst, None
            wsc = small_pool.tile([128, 9], f32, tag="wsc")
            nc.vector.tensor_scalar_mul(wsc[:], wsp[:], a_sc[:])
            return a_sc, bconst, wsc

        NQ = 4          # x DMA slices per tile
        QROWS = H // NQ

        # shared junk target for the ACT Square passes of tiles that have
        # no xpb2 buffer of their own (the PE tile)
        sq_junk = singles.tile([128, NPIX], bf16, tag="sqjunk")

        OUT0, OLEN = 68, 4224
        ALL_TAPS = [(dh, dw) for dh in (-1, 0, 1) for dw in (-1, 0, 1)]

        def pe_warmup(n_mm=14):
            """Junk matmuls at t~6us: the PE HAM clock-gate needs ~3.4us of
            sustained busy to lift K=4/8 -> 8/8.  Running these during the
            (otherwise PE-idle) initial x DMA window means the real depthwise
            matmuls start at full clock instead of paying ~2x for 5us."""
            wjunk = singles.tile([128, 512], bf16, tag="wjunk")
            nc.vector.memset(wjunk[:], 0.0)
            wps = psum_pool.tile([128, 512], f32, tag="ps", name="ps_warm")
            for _ in range(n_mm):
                nc.tensor.matmul(wps[:], wjunk[:, 0:128], wjunk[:],
                                 start=True, stop=True)

        def prefetch_x(b, ct, wsp_early=False):
            """Issue the x-slice + per-tile weight DMAs as early as possible.
            DMA issues serialize at ~0.65us each on the Sync engine, so
            emission order here IS the issue order that decides when each
            tile's data lands.  wsp_early (batch-0 tile 0): wsp right after
            the first x slice so diagw can be built before the x converts
            finish -- the first depthwise matmul needs both."""
            xf = xin_pool.tile([128, H, W], bf16, tag="xf")
            wsp = small_pool.tile([128, 9], f32, tag="wsp")
            for q in range(NQ):
                nc.sync.dma_start(
                    out=xf[:, q * QROWS : (q + 1) * QROWS],
                    in_=x_ext[b, ct * 128 : (ct + 1) * 128, q * QROWS : (q + 1) * QROWS],
                )
                if q == 0 and wsp_early:
                    nc.sync.dma_start(out=wsp[:], in_=wsp_ext[b, ct])
            if not wsp_early:
                nc.sync.dma_start(out=wsp[:], in_=wsp_ext[b, ct])
            wpt = small_pool.tile([128, 1], f32, tag="wpt")
            nc.sync.dma_start(out=wpt[:], in_=wpt_ext[b, ct, :, None])
            bis = small_pool.tile([128, 1], f32, tag="bis")
            nc.sync.dma_start(out=bis[:], in_=bis_ext[b, ct, :, None])
            return {"xf": xf, "wsp": wsp, "wpt": wpt, "bis": bis}

        def shift_copy(xpb, xpb2, chunks=3):
            """xpb2[i] = xpb[i+1] on ACT, split into chunks: a single 4358-el
            pass is ~3.9us of ACT head-of-line blocking for whatever gets
            scheduled behind it (border fills, drains); ~1.3us chunks bound
            that."""
            n = FLAT - 2
            step = -(-n // chunks)
            s = 0
            while s < n:
                e = min(n, s + step)
                nc.scalar.copy(out=xpb2[:, s:e], in_=xpb[:, s + 1 : e + 1])
                s = e
            nc.vector.memset(xpb2[:, FLAT - 2 : FLAT], 0.0)

        def square_passes(pre, sumsq, junk):
            """sum(x^2) via ACT Square; junk output target."""
            xff = pre["xf"][:].rearrange("p a b -> p (a b)")
            for hh in range(2):
                lo = hh * (NPIX // 2)
                hi = lo + NPIX // 2
                nc.scalar.activation(
                    out=junk[:, lo:hi], in_=xff[:, lo:hi],
                    func=AF.Square, accum_out=sumsq[:, hh : hh + 1],
                )

        def convert_passes(pre, xpb, sumx, dve_borders, sumsq, xpb2):
            """grid placement + sum(x) on ACT; sum(x^2) after unless
            sumsq is None (caller emits square_passes itself, later)."""
            xf = pre["xf"]
            for q in range(NQ):
                nc.scalar.activation(
                    out=grid(xpb[:])[:, 1 + q * QROWS : 1 + (q + 1) * QROWS, 1 : 1 + W],
                    in_=xf[:, q * QROWS : (q + 1) * QROWS],
                    func=AF.Copy, accum_out=sumx[:, q : q + 1],
                )
                if dve_borders:
                    g = grid(xpb[:])
                    r0 = 1 + q * QROWS
                    nc.vector.tensor_copy(out=g[:, r0 : r0 + QROWS, 0:1],
                                          in_=g[:, r0 : r0 + QROWS, 2:3])
                    nc.vector.tensor_copy(out=g[:, r0 : r0 + QROWS, 65:66],
                                          in_=g[:, r0 : r0 + QROWS, 63:64])
                    if q == 0:
                        nc.vector.tensor_copy(out=g[:, 0], in_=g[:, 2])
                    if q == NQ - 1:
                        nc.vector.tensor_copy(out=g[:, HPAD - 1], in_=g[:, HPAD - 3])
            if sumsq is not None:
                square_passes(pre, sumsq, xpb2)

        def produce_yp_pe(b, ct, pre):
            """tile (0,0): depthwise on the (still idle) PE via diagonal
            matmuls with UNSCALED taps (no stats dependency); a_sc/bconst
            fold into the PSUM drains.  Drains for the first 2 banks run on
            DVE (idle) so the PSUM pool never stalls the PE; the rest are
            deferred (returned as a finisher) behind tile (0,1)'s ACT ops."""
            wsp, wpt, bis = pre["wsp"], pre["wpt"], pre["bis"]
            xpb = xpb_pool.tile([128, FLAT], bf16, tag="xpb")
            nc.vector.memset(xpb[:, 0:MARG], 0.0)
            nc.vector.memset(xpb[:, FLAT - 2 : FLAT], 0.0)
            sumx = small_pool.tile([128, NQ], f32, tag="sumx")
            sumsq = small_pool.tile([128, 2], f32, tag="sumsq")
            convert_passes(pre, xpb, sumx, dve_borders=True,
                           sumsq=sumsq, xpb2=sq_junk)

            yp = yp_pool.tile([128, FLAT], bf16, tag="yp")
            diagw = small_pool.tile([128, 9, 128], bf16, tag="diagw")
            for t in range(9):
                nc.vector.tensor_scalar_mul(
                    diagw[:, t, :], ident[:], wsp[:, t : t + 1])
            a_sc, bconst, _ = stats_finalize(sumx, sumsq, wsp, wpt, bis,
                                             scale_taps=False)
            banks = []
            s = OUT0
            while s < OUT0 + OLEN:
                n = min(512, OUT0 + OLEN - s)
                ps = psum_pool.tile([128, 512], f32, tag="ps",
                                    name=f"psdw_{b}_{ct}_{s}")
                for t, (dh, dw) in enumerate(ALL_TAPS):
                    toff = WPAD * dh + dw
                    nc.tensor.matmul(
                        ps[:, :n], diagw[:, t, :], xpb[:, s + toff : s + toff + n],
                        start=(t == 0), stop=(t == 8),
                    )
                banks.append((s, n, ps))
                if len(banks) <= 2:
                    # first two drains emitted inline (the scheduler runs
                    # them early): frees PSUM slot 0/1 before the 9th bank
                    # allocation needs one.  dw(0,0) drains run on DVE --
                    # their stats dep clears at ~20us, before the (0,1)
                    # tree dominates the DVE queue, and keeping them off
                    # ACT avoids pushing batch-1's converts later (which
                    # showed up as a 7.5us b0->b1 stall).
                    s_, n_, ps_ = banks[-1]
                    nc.vector.tensor_scalar(
                        yp[:, s_ : s_ + n_], ps_[:, :n_], a_sc[:], bconst[:],
                        op0=ALU.mult, op1=ALU.add,
                    )
                s += n
            yp_tiles[(b, ct)] = yp

            def finish():
                for s_, n_, ps_ in banks[2:]:
                    nc.vector.tensor_scalar(
                        yp[:, s_ : s_ + n_], ps_[:, :n_], a_sc[:], bconst[:],
                        op0=ALU.mult, op1=ALU.add,
                    )
                nc.vector.memset(yp[:, 0:MARG], 0.0)
                nc.vector.memset(yp[:, FLAT - 2 : FLAT], 0.0)
                fill_borders(yp)

            return finish

        def tap_idx(dh, dw):
            return (dh + 1) * 3 + (dw + 1)

        def tap_src_f(xpb, xpb2):
            def tap_src(dh, dw):
                t_off = WPAD * dh + dw
                if dw == 0:
                    s = OUT0 + t_off       # even
                    return xpb[:, s : s + OLEN]
                s = OUT0 - 1 + t_off       # even (t_off odd)
                return xpb2[:, s : s + OLEN]
            return tap_src

        # tile (0,1) depthwise is split: the DVE tree covers the first
        # DW_SPLIT flat elements; the PE (which would otherwise idle ~9us
        # waiting for the tree) computes the tail segments via diagonal
        # matmuls, exactly like tile (0,0).  2560 (down from 3072) trades
        # ~4us of DVE-tree time for ~2 more PE banks: yp(0,1) is the gate
        # on big_conv(0)'s ct1 pass and the PE otherwise has a hole there.
        DW_SPLIT = 2560

        def produce_yp_early(b, ct, pre, post_stats_hook):
            """tile (0,1): latency-critical variant.  Taps use the RAW wsp
            weights (no stats dependency -- a_sc/bconst apply in one final
            rescale pass / fold into the PE-part PSUM drains), and TWO taps
            run on ACT.  post_stats_hook emits tile (0,0)'s deferred drains
            right after the shift copy so they don't delay the tree."""
            wsp, wpt, bis = pre["wsp"], pre["wpt"], pre["bis"]
            # diagonal weights for the PE part first: only needs wsp (early)
            diagw = small_pool.tile([128, 9, 128], bf16, tag="diagw")
            for t in range(9):
                nc.vector.tensor_scalar_mul(
                    diagw[:, t, :], ident[:], wsp[:, t : t + 1])
            xpb = xpb_pool.tile([128, FLAT], bf16, tag="xpb")
            xpb2 = xpb2_pool.tile([128, FLAT], bf16, tag="xpb2")
            nc.vector.memset(xpb[:, 0:MARG], 0.0)
            nc.vector.memset(xpb[:, FLAT - 2 : FLAT], 0.0)
            sumx = small_pool.tile([128, NQ], f32, tag="sumx")
            sumsq = small_pool.tile([128, 2], f32, tag="sumsq")
            # squares DEFERRED to after the shift copy: the tree's odd-tap
            # muls only need xpb2 (shift), and stats are only needed by the
            # final rescale pass -- this starts the DVE tree ~4us earlier.
            convert_passes(pre, xpb, sumx, dve_borders=False,
                           sumsq=None, xpb2=None)
            fill_borders(xpb)
            shift_copy(xpb, xpb2)

            if post_stats_hook is not None:
                post_stats_hook()
            square_passes(pre, sumsq, sq_junk)
            a_sc, bconst, _ = stats_finalize(sumx, sumsq, wsp, wpt, bis,
                                             scale_taps=False)

            yp = yp_pool.tile([128, FLAT], bf16, tag="yp")

            # ---- PE part: tail segments [OUT0+DW_SPLIT, OUT0+OLEN) ----
            pe_banks = []
            s = OUT0 + DW_SPLIT
            while s < OUT0 + OLEN:
                n = min(512, OUT0 + OLEN - s)
                ps = psum_pool.tile([128, 512], f32, tag="ps",
                                    name=f"psdw_{b}_{ct}_{s}")
                for t, (dh, dw) in enumerate(ALL_TAPS):
                    toff = WPAD * dh + dw
                    nc.tensor.matmul(
                        ps[:, :n], diagw[:, t, :], xpb[:, s + toff : s + toff + n],
                        start=(t == 0), stop=(t == 8),
                    )
                pe_banks.append((s, n, ps))
                s += n
            for s_, n_, ps_ in pe_banks:
                # DVE drain (see produce_yp_pe): ACT is saturated at b0
                nc.vector.tensor_scalar(
                    yp[:, s_ : s_ + n_], ps_[:, :n_], a_sc[:], bconst[:],
                    op0=ALU.mult, op1=ALU.add,
                )

            # ---- DVE tree part: [OUT0, OUT0+DW_SPLIT) ----
            L = DW_SPLIT
            yp_seg = yp[:, OUT0 : OUT0 + L]
            tap_src0 = tap_src_f(xpb, xpb2)

            def tap_src(dh, dw):
                return tap_src0(dh, dw)[:, :L]

            # ACT taps: the two even taps (0,0) and (1,0) (xpb-only reads)
            act_taps = [(0, 0), (1, 0)]
            tmpa = tmp_pool.tile([128, OLEN], bf16, tag="dwtmpa")
            nc.scalar.mul(tmpa[:, :L], tap_src(0, 0), wsp[:, tap_idx(0, 0) : tap_idx(0, 0) + 1])
            tmpd = tmp_pool.tile([128, OLEN], bf16, tag="dwtmpd", bufs=1, name=f"dwtmpd_{b}_{ct}")
            nc.scalar.mul(tmpd[:, :L], tap_src(1, 0), wsp[:, tap_idx(1, 0) : tap_idx(1, 0) + 1])

            d0 = (-1, 0)  # remaining even tap, on DVE, xpb-only
            rest = [t for t in ALL_TAPS if t not in act_taps and t != d0]  # 6 odd taps

            def mul_into(buf, tap):
                t = tap_idx(*tap)
                nc.vector.tensor_scalar_mul(buf[:, :L], tap_src(*tap), wsp[:, t : t + 1])

            ta = tmp_pool.tile([128, OLEN], bf16, tag="dwA", bufs=1, name=f"dwA_{b}_{ct}")
            tb = tmp_pool.tile([128, OLEN], bf16, tag="dwB", bufs=1, name=f"dwB_{b}_{ct}")
            tc_ = tmp_pool.tile([128, OLEN], bf16, tag="dwC", bufs=1, name=f"dwC_{b}_{ct}")
            mul_into(ta, rest[0])
            mul_into(tb, rest[1])
            nc.vector.tensor_add(ta[:, :L], ta[:, :L], tb[:, :L])
            mul_into(tb, rest[2])
            mul_into(tc_, rest[3])
            nc.vector.tensor_add(tb[:, :L], tb[:, :L], tc_[:, :L])
            nc.vector.tensor_add(ta[:, :L], ta[:, :L], tb[:, :L])   # 4 odd taps
            mul_into(tb, rest[4])
            mul_into(tc_, rest[5])
            nc.vector.tensor_add(tb[:, :L], tb[:, :L], tc_[:, :L])
            nc.vector.tensor_add(ta[:, :L], ta[:, :L], tb[:, :L])   # all 6 odd taps
            mul_into(tb, d0)
            nc.vector.tensor_add(tmpa[:, :L], tmpa[:, :L], tmpd[:, :L])  # ACT pair
            nc.vector.tensor_add(tb[:, :L], tb[:, :L], tmpa[:, :L])
            nc.vector.tensor_add(ta[:, :L], ta[:, :L], tb[:, :L])   # u = all 9 taps
            # final rescale (out-of-place): yp = u * a_sc + bconst
            nc.vector.tensor_scalar(
                yp_seg, ta[:, :L], a_sc[:], bconst[:], op0=ALU.mult, op1=ALU.add)

            nc.vector.memset(yp[:, 0:MARG], 0.0)
            nc.vector.memset(yp[:, FLAT - 2 : FLAT], 0.0)
            fill_borders(yp)
            yp_tiles[(b, ct)] = yp

        def produce_yp_steady(b, ct, pre, act2=False):
            """norm + depthwise pipeline, steady-state variant.  act2: run
            TWO taps on ACT instead of one -- used for the ct=1 tile whose
            completion gates the big-conv ct1 pass ~15.5us into each batch
            (DVE is the tighter engine there; ACT has slack)."""
            wsp, wpt, bis = pre["wsp"], pre["wpt"], pre["bis"]
            xpb = xpb_pool.tile([128, FLAT], bf16, tag="xpb")
            xpb2 = xpb2_pool.tile([128, FLAT], bf16, tag="xpb2")
            nc.vector.memset(xpb[:, 0:MARG], 0.0)
            nc.vector.memset(xpb[:, FLAT - 2 : FLAT], 0.0)
            sumx = small_pool.tile([128, NQ], f32, tag="sumx")
            sumsq = small_pool.tile([128, 2], f32, tag="sumsq")
            convert_passes(pre, xpb, sumx, dve_borders=False,
                           sumsq=sumsq, xpb2=xpb2)
            fill_borders(xpb)

            # shifted copy (one element left) for 4B-aligned odd-offset
            # taps (DVE bf16 2x mode). NOTE: gpsimd bulk ops are poison here
            # -- they hold the shared DVE/GpSimd SBUF port for their whole
            # duration and stall every DVE tensor_tensor op; stays on ACT.
            shift_copy(xpb, xpb2)

            _, bconst, wsc = stats_finalize(sumx, sumsq, wsp, wpt, bis,
                                            scale_taps=True)

            yp = yp_pool.tile([128, FLAT], bf16, tag="yp")
            yp_seg = yp[:, OUT0 : OUT0 + OLEN]
            tap_src = tap_src_f(xpb, xpb2)

            # center tap's multiply runs on ACT (it has slack); the other
            # taps' products come from DVE 2x-mode tensor_scalar muls, then
            # are combined with a pairwise ADD TREE (same op count as a
            # serial chain but 4x shorter dependency depth and ~2x better
            # bf16 rounding error).  ACT muls are split in halves so they
            # never block the ACT queue for >2us.
            def act_mul(buf, tap):
                t = tap_idx(*tap)
                h = OLEN // 2
                nc.scalar.mul(buf[:, :h], tap_src(*tap)[:, :h],
                              wsc[:, t : t + 1])
                nc.scalar.mul(buf[:, h:], tap_src(*tap)[:, h:],
                              wsc[:, t : t + 1])

            tmpa = tmp_pool.tile([128, OLEN], bf16, tag="dwtmpa")
            act_mul(tmpa, (0, 0))

            # tap0 writes yp_seg = w0*x0 + B directly
            d0, w0 = ALL_TAPS[0]
            t0 = tap_idx(d0, w0)
            nc.vector.tensor_scalar(
                yp_seg, tap_src(d0, w0), wsc[:, t0 : t0 + 1], bconst[:],
                op0=ALU.mult, op1=ALU.add,
            )

            def mul_into(buf, tap):
                t = tap_idx(*tap)
                nc.vector.tensor_scalar_mul(buf[:], tap_src(*tap), wsc[:, t : t + 1])

            ta = tmp_pool.tile([128, OLEN], bf16, tag="dwA", bufs=1, name=f"dwA_{b}_{ct}")
            tb = tmp_pool.tile([128, OLEN], bf16, tag="dwB", bufs=1, name=f"dwB_{b}_{ct}")
            tc_ = tmp_pool.tile([128, OLEN], bf16, tag="dwC", bufs=1, name=f"dwC_{b}_{ct}")
            if act2:
                # second ACT tap (1,0): DVE drops to 6 muls + 8 adds
                tmpd = tmp_pool.tile([128, OLEN], bf16, tag="dwtmpd",
                                     bufs=1, name=f"dwtmpd_{b}_{ct}")
                act_mul(tmpd, (1, 0))
                rest = [t for t in ALL_TAPS[1:] if t not in ((0, 0), (1, 0))]
                mul_into(ta, rest[0])
                mul_into(tb, rest[1])
                nc.vector.tensor_add(ta[:], ta[:], tb[:])
                mul_into(tb, rest[2])
                mul_into(tc_, rest[3])
                nc.vector.tensor_add(tb[:], tb[:], tc_[:])
                nc.vector.tensor_add(ta[:], ta[:], tb[:])    # 4 DVE taps
                mul_into(tb, rest[4])
                mul_into(tc_, rest[5])
                nc.vector.tensor_add(tb[:], tb[:], tc_[:])
                nc.vector.tensor_add(tc_[:], tmpa[:], tmpd[:])  # ACT pair
                nc.vector.tensor_add(tb[:], tb[:], tc_[:])
                nc.vector.tensor_add(yp_seg, yp_seg, ta[:])
                nc.vector.tensor_add(yp_seg, yp_seg, tb[:])
            else:
                rest = [t for t in ALL_TAPS[1:] if t != (0, 0)]  # 7 taps
                mul_into(ta, rest[0])
                mul_into(tb, rest[1])
                nc.vector.tensor_add(ta[:], ta[:], tb[:])
                mul_into(tb, rest[2])
                mul_into(tc_, rest[3])
                nc.vector.tensor_add(tb[:], tb[:], tc_[:])
                nc.vector.tensor_add(ta[:], ta[:], tb[:])        # taps 1-4
                mul_into(tb, rest[4])
                mul_into(tc_, rest[5])
                nc.vector.tensor_add(tb[:], tb[:], tc_[:])
                mul_into(tc_, rest[6])
                nc.vector.tensor_add(tc_[:], tc_[:], tmpa[:])    # + ACT tap
                nc.vector.tensor_add(tb[:], tb[:], tc_[:])       # taps 5-7 + act
                nc.vector.tensor_add(yp_seg, yp_seg, ta[:])
                nc.vector.tensor_add(yp_seg, yp_seg, tb[:])

            nc.vector.memset(yp[:, 0:MARG], 0.0)
            nc.vector.memset(yp[:, FLAT - 2 : FLAT], 0.0)
            fill_borders(yp)
            yp_tiles[(b, ct)] = yp

        # tap order for the ct-outer paths: center tap first -- its rhs
        # reads only interior yp cells, so the bank-start matmuls don't
        # wait on the border-fill ACT ops.
        CT_OUTER_TAPS = [(0, 0)] + [
            (dh, dw) for dh in (-1, 0, 1) for dw in (-1, 0, 1) if (dh, dw) != (0, 0)
        ]

        def drain_bank(b, ot, stage, r0, nr, p, out_slice):
            src = p[:].rearrange("p (r c) -> p r c", c=W)
            nc.scalar.activation(
                out=stage[:, r0 : r0 + nr, :], in_=src,
                func=AF.Identity, bias=cb_sb[:, ot : ot + 1],
            )
            rend = r0 + nr
            if rend % out_slice == 0:
                s0 = rend - out_slice
                nc.sync.dma_start(
                    out=out_ext[b, ot * 128 : (ot + 1) * 128, s0:rend],
                    in_=stage[:, s0:rend],
                )

        def big_conv(b):
            """Structure (per ot): for ot0 of batches>0, the ct0 taps run
            tap-outer first, giving PE ~15.5us of runway on yp[b,0] alone
            while DVE finishes yp[b,1].  Everything else runs BANK-OUTER
            with an immediate per-bank drain, so PSUM banks recycle
            continuously -- the old all-8-banks-finish-at-once shape made
            every ot/batch transition stall ~2-3us on queued ACT drains."""
            last = b == nb - 1
            for ot in range(OT):
                stage = stage_pool.tile([128, H, W], bf16, tag="stage")
                # 8-row slices for the final ot so the very last out DMA
                # (which the kernel-end barrier waits on) is small
                out_slice = 8 if (last and ot == OT - 1) else OUT_SLICE
                ps = {}
                for r0, nr in ROW_BLOCKS:
                    ps[r0] = psum_pool.tile(
                        [128, BLK_ROWS * W], f32, tag="ps",
                        name=f"ps_{b}_{ot}_{r0}",
                    )
                runway = ot == 0 and b > 0
                if runway:
                    ypg = grid(yp_tiles[(b, 0)][:])
                    for ti, (dh, dw) in enumerate(CT_OUTER_TAPS):
                        lhsT = cw_sb[0][:, dh + 1, dw + 1, ot, :]
                        for r0, nr in ROW_BLOCKS:
                            rhs = ypg[:, r0 + 1 + dh : r0 + 1 + dh + nr,
                                      1 + dw : 1 + dw + W]
                            nc.tensor.matmul(ps[r0][:], lhsT, rhs,
                                             start=(ti == 0), stop=False)
                rest_cts = [1] if runway else list(range(CT))
                n_acc = len(rest_cts) * 9
                for r0, nr in ROW_BLOCKS:
                    i = 0
                    for ct in rest_cts:
                        ypg = grid(yp_tiles[(b, ct)][:])
                        for dh, dw in CT_OUTER_TAPS:
                            lhsT = cw_sb[ct][:, dh + 1, dw + 1, ot, :]
                            rhs = ypg[:, r0 + 1 + dh : r0 + 1 + dh + nr,
                                      1 + dw : 1 + dw + W]
                            nc.tensor.matmul(
                                ps[r0][:], lhsT, rhs,
                                start=(not runway and i == 0),
                                stop=(i == n_acc - 1),
                            )
                            i += 1
                    drain_bank(b, ot, stage, r0, nr, ps[r0], out_slice)

        # Emission order doubles as DMA-issue order (Sync engine serializes
        # issues at ~0.65us each): batch 0's x slices go absolutely first,
        # then the weights, then each later batch's x prefetches interleave
        # ahead of the previous batch's big_conv.
        pe_warmup()
        pre = {}
        pre[(0, 0)] = prefetch_x(0, 0, wsp_early=True)
        pre[(0, 1)] = prefetch_x(0, 1)
        dw_finish = produce_yp_pe(0, 0, pre[(0, 0)])
        load_cw()
        produce_yp_early(0, 1, pre[(0, 1)], post_stats_hook=dw_finish)
        for b in range(nb):
            if b + 1 < nb:
                pre[(b + 1, 0)] = prefetch_x(b + 1, 0)
                pre[(b + 1, 1)] = prefetch_x(b + 1, 1)
            big_conv(b)
            if b + 1 < nb:
                produce_yp_steady(b + 1, 0, pre[(b + 1, 0)])
                produce_yp_steady(b + 1, 1, pre[(b + 1, 1)], act2=True)

    nc.compile()
    return nc


def _host_prep(x, w_spatial, w_pointwise, bias, conv_w, conv_b, nb=NB):
    import ml_dtypes

    ncores = x.shape[0] // nb
    cw = np.ascontiguousarray(
        conv_w.reshape(OT, 128, CT, 128, 3, 3).transpose(2, 3, 4, 5, 0, 1)
    ).astype(ml_dtypes.bfloat16)
    cb = np.ascontiguousarray(conv_b.reshape(OT, 128)).astype(np.float32)
    wsp = np.ascontiguousarray(w_spatial.reshape(-1, CT, 128, 9)).astype(np.float32)
    wpt = np.ascontiguousarray(w_pointwise.reshape(-1, CT, 128)).astype(np.float32)
    bis = np.ascontiguousarray(bias.reshape(-1, CT, 128)).astype(np.float32)
    x = np.ascontiguousarray(x).astype(ml_dtypes.bfloat16)
    in_maps = []
    for i in range(ncores):
        sl = slice(i * nb, (i + 1) * nb)
        in_maps.append({
            "x": np.ascontiguousarray(x[sl]),
            "wsp": np.ascontiguousarray(wsp[sl]),
            "wpt": np.ascontiguousarray(wpt[sl]),
            "bis": np.ascontiguousarray(bis[sl]),
            "cw": cw,
            "cb": cb,
        })
    return in_maps


def _run(inputs, trace=False):
    from concourse.bass_utils import run_bass_kernel_spmd

    if "nc" not in _CACHED:
        _CACHED["nc"] = _build()
    nc = _CACHED["nc"]
    in_maps = _host_prep(**inputs)
    kw = {}
    if trace:
        import shutil
        tdir = "/tmp/kernel_trace_out"
        shutil.rmtree(tdir, ignore_errors=True)
        os.makedirs(tdir, exist_ok=True)
        kw["tmpdir"] = tdir
    res = run_bass_kernel_spmd(
        nc, in_maps, core_ids=list(range(N_CORES)), trace=trace, **kw
    )
    out = np.concatenate([res.results[i]["out"] for i in range(N_CORES)], axis=0)
    return out.astype(np.float32), res


def kernel(x, w_spatial, w_pointwise, bias, conv_w, conv_b):
    out, _ = _run(
        dict(x=np.asarray(x), w_spatial=np.asarray(w_spatial),
             w_pointwise=np.asarray(w_pointwise), bias=np.asarray(bias),
             conv_w=np.asarray(conv_w), conv_b=np.asarray(conv_b)),
        trace=bool(int(os.environ.get("KERNEL_TRACE", "0"))),
    )
    return out



# revision 63
# speedup vs baseline: 1.1946x; 1.1946x over previous
"""AdaConv2d fused kernel for 8 TRN2 NeuronCores (pure data parallel).

Per-sample pipeline (all fused on-chip):
  1. instance-norm stats (mean/var over HW)
  2. dynamic per-(b,c) depthwise 3x3 conv with reflect padding
  3. per-(b,c) scale+bias (folded algebraically into the depthwise taps:
     y = A*(sum_t w_t * x_t) + B with A = rstd*w_pt, B = bias - mu*A*sum(w))
  4. fixed 3x3 conv (256->256) with reflect padding, as 18 accumulated
     bf16 matmuls per PSUM block

Layout: channels on partitions (2 tiles of 128), pixels on the free axis.
Padded images are 66 rows x 66 cols stored flat with a 2-element leading
margin (so every depthwise tap and every matmul rhs is a fully CONTIGUOUS
1D slice).  flat(r, c) = 2 + 66*r + c.  Rows 0/65 and cols 0/65 are the
reflect pads.  A one-element-left-shifted copy (xpb2[i] = xpb[i+1]) keeps
all odd-offset depthwise taps 4-byte aligned for the DVE bf16 2x mode.

Perf notes (measured on hw):
  - bf16 matmul N=512 paces at ~216 ns warm; 3D-AP rhs costs ~nothing.
    Slow stretches in traces are PE p-state/DVFS ramp, not AP shape.
  - DVE: TS muls ~2x (1.25us/4224), TT adds 1x (2.35us), STT and custom
    DVE ops 0.5x (4.6us) -> the TS-mul + TT-add-tree depthwise is optimal.
  - fp8 DoubleRow matmul paces 427ns for K=256 = zero gain over 2x bf16.
  - First-batch critical path: x DMA in 4 slices, ACT converts first,
    PE depthwise uses UNSCALED taps (stats fold into the PSUM drain) so
    the first matmul doesn't wait for stats.
"""

import os
from contextlib import ExitStack

import numpy as np

B_GLOBAL = 32
N_CORES = 8
NB = B_GLOBAL // N_CORES  # batches per core
C = 256
H = W = 64
WPAD = W + 2        # 66 padded row length
HPAD = H + 2        # 66 padded rows
MARG = 2            # leading margin so tap windows stay in-bounds
FLAT = MARG + HPAD * WPAD + 2   # 4360 flat elements per padded image
NPIX = H * W        # 4096
CT = C // 128       # channel tiles
OT = C // 128       # out-channel tiles
EPS = 1e-5
BLK_ROWS = 8        # output rows per PSUM block (8*64=512 fp32, 3D-AP rhs)

ROW_BLOCKS = [(r0, BLK_ROWS) for r0 in range(0, H, BLK_ROWS)]
OUT_SLICE = 16      # rows per output DMA slice (tail-latency hiding)

_CACHED = {}


def _build(nb=NB):
    import concourse.mybir as mybir
    import concourse.tile as tile
    from concourse import bacc

    f32 = mybir.dt.float32
    bf16 = mybir.dt.bfloat16
    AF = mybir.ActivationFunctionType
    ALU = mybir.AluOpType

    nc = bacc.Bacc(None, target_bir_lowering=False)

    # x and out travel as bf16 (host converts): halves both big DMA streams;
    # measured end-to-end numeric impact ~3.2e-3 rel err (vs 2e-2 budget).
    x_ext = nc.declare_dram_parameter("x", [nb, C, H, W], bf16, isOutput=False)
    wsp_ext = nc.declare_dram_parameter("wsp", [nb, CT, 128, 9], f32, isOutput=False)
    wpt_ext = nc.declare_dram_parameter("wpt", [nb, CT, 128], f32, isOutput=False)
    bis_ext = nc.declare_dram_parameter("bis", [nb, CT, 128], f32, isOutput=False)
    cw_ext = nc.declare_dram_parameter("cw", [CT, 128, 3, 3, OT, 128], bf16, isOutput=False)
    cb_ext = nc.declare_dram_parameter("cb", [OT, 128], f32, isOutput=False)
    out_ext = nc.declare_dram_parameter("out", [nb, C, H, W], bf16, isOutput=True)

    with tile.TileContext(nc) as tc, ExitStack() as ctx:
        singles = ctx.enter_context(tc.tile_pool(name="singles", bufs=1))
        xin_pool = ctx.enter_context(tc.tile_pool(name="xin", bufs=4))
        xpb_pool = ctx.enter_context(tc.tile_pool(name="xpb", bufs=2))
        xpb2_pool = ctx.enter_context(tc.tile_pool(name="xpb2", bufs=2))
        yp_pool = ctx.enter_context(tc.tile_pool(name="yp", bufs=4))
        stage_pool = ctx.enter_context(tc.tile_pool(name="stage", bufs=2))
        small_pool = ctx.enter_context(tc.tile_pool(name="small", bufs=6))
        tmp_pool = ctx.enter_context(tc.tile_pool(name="tmp", bufs=2))
        psum_pool = ctx.enter_context(tc.tile_pool(name="psum", bufs=8, space="PSUM"))

        # ---- constants / fixed weights ----
        # (emitted lazily AFTER batch 0's x DMAs so the 1.2MB weight load
        # doesn't delay the first tile's data; weights are only needed at
        # ~40us when the first big-conv matmul fires)
        cw_sb = []

        cb_sb = singles.tile([128, OT], f32, tag="cb")

        def load_cw():
            for ct in range(CT):
                t = singles.tile([128, 3, 3, OT, 128], bf16, tag=f"cw{ct}")
                nc.sync.dma_start(out=t[:], in_=cw_ext[ct])
                cw_sb.append(t)
            for ot in range(OT):
                nc.sync.dma_start(out=cb_sb[:, ot : ot + 1], in_=cb_ext[ot, :, None])
        eps_sb = singles.tile([128, 1], f32, tag="eps")
        nc.vector.memset(eps_sb[:], EPS)
        ident = singles.tile([128, 128], bf16, tag="ident")
        from concourse.masks import make_identity
        make_identity(nc, ident[:])
        # touch the Sqrt activation table once so its ~1.3us load happens
        # before the first tile's stats need it
        warm = singles.tile([128, 1], f32, tag="warm")
        nc.scalar.activation(out=warm[:], in_=eps_sb[:], func=AF.Sqrt, bias=eps_sb[:])

        yp_tiles = {}

        def grid(flat_ap):
            """(128, FLAT) flat padded buffer -> (128, 66, 66) image view."""
            return flat_ap[:, MARG : MARG + HPAD * WPAD].rearrange(
                "p (r c) -> p r c", c=WPAD)

        def fill_borders(buf):
            g = grid(buf[:])
            nc.scalar.copy(out=g[:, 1 : 1 + H, 0:1], in_=g[:, 1 : 1 + H, 2:3])
            nc.scalar.copy(out=g[:, 1 : 1 + H, 65:66], in_=g[:, 1 : 1 + H, 63:64])
            nc.scalar.copy(out=g[:, 0], in_=g[:, 2])
            nc.scalar.copy(out=g[:, HPAD - 1], in_=g[:, HPAD - 3])

        def stats_finalize(sumx, sumsq, wsp, wpt, bis, scale_taps):
            """Common stats tail: returns (a_sc, bconst, wsc-or-None)."""
            sx = small_pool.tile([128, 1], f32, tag="sx")
            nc.vector.reduce_sum(sx[:], sumx[:], axis=mybir.AxisListType.X)
            sq = small_pool.tile([128, 1], f32, tag="sq")
            nc.vector.reduce_sum(sq[:], sumsq[:], axis=mybir.AxisListType.X)
            mu = small_pool.tile([128, 1], f32, tag="mu")
            nc.vector.tensor_scalar_mul(mu[:], sx[:], 1.0 / NPIX)
            m2 = small_pool.tile([128, 1], f32, tag="m2")
            nc.vector.tensor_scalar_mul(m2[:], sq[:], 1.0 / NPIX)
            musq = small_pool.tile([128, 1], f32, tag="musq")
            nc.vector.tensor_mul(musq[:], mu[:], mu[:])
            var = small_pool.tile([128, 1], f32, tag="var")
            nc.vector.tensor_sub(var[:], m2[:], musq[:])
            std = small_pool.tile([128, 1], f32, tag="std")
            nc.scalar.activation(out=std[:], in_=var[:], func=AF.Sqrt, bias=eps_sb[:])
            rstd = small_pool.tile([128, 1], f32, tag="rstd")
            nc.vector.reciprocal(out=rstd[:], in_=std[:])
            a_sc = small_pool.tile([128, 1], f32, tag="a_sc")
            nc.vector.tensor_mul(a_sc[:], rstd[:], wpt[:])
            sw = small_pool.tile([128, 1], f32, tag="sw")
            nc.vector.reduce_sum(sw[:], wsp[:], axis=mybir.AxisListType.X)
            t1 = small_pool.tile([128, 1], f32, tag="t1")
            nc.vector.tensor_mul(t1[:], mu[:], a_sc[:])
            nc.vector.tensor_mul(t1[:], t1[:], sw[:])
            bconst = small_pool.tile([128, 1], f32, tag="bconst")
            nc.vector.tensor_sub(bconst[:], bis[:], t1[:])
            if not scale_taps:
                return a_sc, bconst, None
            wsc = small_pool.tile([128, 9], f32, tag="wsc")
            nc.vector.tensor_scalar_mul(wsc[:], wsp[:], a_sc[:])
            return a_sc, bconst, wsc

        NQ = 4          # x DMA slices per tile
        QROWS = H // NQ

        # shared junk target for the ACT Square passes of tiles that have
        # no xpb2 buffer of their own (the PE tile)
        sq_junk = singles.tile([128, NPIX], bf16, tag="sqjunk")

        OUT0, OLEN = 68, 4224
        ALL_TAPS = [(dh, dw) for dh in (-1, 0, 1) for dw in (-1, 0, 1)]

        def pe_warmup(n_mm=14):
            """Junk matmuls at t~6us: the PE HAM clock-gate needs ~3.4us of
            sustained busy to lift K=4/8 -> 8/8.  Running these during the
            (otherwise PE-idle) initial x DMA window means the real depthwise
            matmuls start at full clock instead of paying ~2x for 5us."""
            wjunk = singles.tile([128, 512], bf16, tag="wjunk")
            nc.vector.memset(wjunk[:], 0.0)
            wps = psum_pool.tile([128, 512], f32, tag="ps", name="ps_warm")
            for _ in range(n_mm):
                nc.tensor.matmul(wps[:], wjunk[:, 0:128], wjunk[:],
                                 start=True, stop=True)

        def prefetch_x(b, ct, wsp_early=False):
            """Issue the x-slice + per-tile weight DMAs as early as possible.
            DMA issues serialize at ~0.65us each on the Sync engine, so
            emission order here IS the issue order that decides when each
            tile's data lands.  wsp_early (batch-0 tile 0): wsp right after
            the first x slice so diagw can be built before the x converts
            finish -- the first depthwise matmul needs both."""
            xf = xin_pool.tile([128, H, W], bf16, tag="xf")
            wsp = small_pool.tile([128, 9], f32, tag="wsp")
            for q in range(NQ):
                nc.sync.dma_start(
                    out=xf[:, q * QROWS : (q + 1) * QROWS],
                    in_=x_ext[b, ct * 128 : (ct + 1) * 128, q * QROWS : (q + 1) * QROWS],
                )
                if q == 0 and wsp_early:
                    nc.sync.dma_start(out=wsp[:], in_=wsp_ext[b, ct])
            if not wsp_early:
                nc.sync.dma_start(out=wsp[:], in_=wsp_ext[b, ct])
            wpt = small_pool.tile([128, 1], f32, tag="wpt")
            nc.sync.dma_start(out=wpt[:], in_=wpt_ext[b, ct, :, None])
            bis = small_pool.tile([128, 1], f32, tag="bis")
            nc.sync.dma_start(out=bis[:], in_=bis_ext[b, ct, :, None])
            return {"xf": xf, "wsp": wsp, "wpt": wpt, "bis": bis}

        def shift_copy(xpb, xpb2, chunks=3):
            """xpb2[i] = xpb[i+1] on ACT, split into chunks: a single 4358-el
            pass is ~3.9us of ACT head-of-line blocking for whatever gets
            scheduled behind it (border fills, drains); ~1.3us chunks bound
            that."""
            n = FLAT - 2
            step = -(-n // chunks)
            s = 0
            while s < n:
                e = min(n, s + step)
                nc.scalar.copy(out=xpb2[:, s:e], in_=xpb[:, s + 1 : e + 1])
                s = e
            nc.vector.memset(xpb2[:, FLAT - 2 : FLAT], 0.0)

        def square_passes(pre, sumsq, junk):
            """sum(x^2) via ACT Square; junk output target."""
            xff = pre["xf"][:].rearrange("p a b -> p (a b)")
            for hh in range(2):
                lo = hh * (NPIX // 2)
                hi = lo + NPIX // 2
                nc.scalar.activation(
                    out=junk[:, lo:hi], in_=xff[:, lo:hi],
                    func=AF.Square, accum_out=sumsq[:, hh : hh + 1],
                )

        def convert_passes(pre, xpb, sumx, dve_borders, sumsq, xpb2):
            """grid placement + sum(x) on ACT; sum(x^2) after unless
            sumsq is None (caller emits square_passes itself, later)."""
            xf = pre["xf"]
            for q in range(NQ):
                nc.scalar.activation(
                    out=grid(xpb[:])[:, 1 + q * QROWS : 1 + (q + 1) * QROWS, 1 : 1 + W],
                    in_=xf[:, q * QROWS : (q + 1) * QROWS],
                    func=AF.Copy, accum_out=sumx[:, q : q + 1],
                )
                if dve_borders:
                    g = grid(xpb[:])
                    r0 = 1 + q * QROWS
                    nc.vector.tensor_copy(out=g[:, r0 : r0 + QROWS, 0:1],
                                          in_=g[:, r0 : r0 + QROWS, 2:3])
                    nc.vector.tensor_copy(out=g[:, r0 : r0 + QROWS, 65:66],
                                          in_=g[:, r0 : r0 + QROWS, 63:64])
                    if q == 0:
                        nc.vector.tensor_copy(out=g[:, 0], in_=g[:, 2])
                    if q == NQ - 1:
                        nc.vector.tensor_copy(out=g[:, HPAD - 1], in_=g[:, HPAD - 3])
            if sumsq is not None:
                square_passes(pre, sumsq, xpb2)

        def produce_yp_pe(b, ct, pre):
            """tile (0,0): depthwise on the (still idle) PE via diagonal
            matmuls with UNSCALED taps (no stats dependency); a_sc/bconst
            fold into the PSUM drains.  Drains for the first 2 banks run on
            DVE (idle) so the PSUM pool never stalls the PE; the rest are
            deferred (returned as a finisher) behind tile (0,1)'s ACT ops."""
            wsp, wpt, bis = pre["wsp"], pre["wpt"], pre["bis"]
            xpb = xpb_pool.tile([128, FLAT], bf16, tag="xpb")
            nc.vector.memset(xpb[:, 0:MARG], 0.0)
            nc.vector.memset(xpb[:, FLAT - 2 : FLAT], 0.0)
            sumx = small_pool.tile([128, NQ], f32, tag="sumx")
            sumsq = small_pool.tile([128, 2], f32, tag="sumsq")
            convert_passes(pre, xpb, sumx, dve_borders=True,
                           sumsq=sumsq, xpb2=sq_junk)

            yp = yp_pool.tile([128, FLAT], bf16, tag="yp")
            diagw = small_pool.tile([128, 9, 128], bf16, tag="diagw")
            for t in range(9):
                nc.vector.tensor_scalar_mul(
                    diagw[:, t, :], ident[:], wsp[:, t : t + 1])
            a_sc, bconst, _ = stats_finalize(sumx, sumsq, wsp, wpt, bis,
                                             scale_taps=False)
            banks = []
            s = OUT0
            while s < OUT0 + OLEN:
                n = min(512, OUT0 + OLEN - s)
                ps = psum_pool.tile([128, 512], f32, tag="ps",
                                    name=f"psdw_{b}_{ct}_{s}")
                for t, (dh, dw) in enumerate(ALL_TAPS):
                    toff = WPAD * dh + dw
                    nc.tensor.matmul(
                        ps[:, :n], diagw[:, t, :], xpb[:, s + toff : s + toff + n],
                        start=(t == 0), stop=(t == 8),
                    )
                banks.append((s, n, ps))
                if len(banks) <= 2:
                    # first two drains emitted inline (the scheduler runs
                    # them early): frees PSUM slot 0/1 before the 9th bank
                    # allocation needs one.  dw(0,0) drains run on DVE --
                    # their stats dep clears at ~20us, before the (0,1)
                    # tree dominates the DVE queue, and keeping them off
                    # ACT avoids pushing batch-1's converts later (which
                    # showed up as a 7.5us b0->b1 stall).
                    s_, n_, ps_ = banks[-1]
                    nc.vector.tensor_scalar(
                        yp[:, s_ : s_ + n_], ps_[:, :n_], a_sc[:], bconst[:],
                        op0=ALU.mult, op1=ALU.add,
                    )
                s += n
            yp_tiles[(b, ct)] = yp

            def finish():
                for s_, n_, ps_ in banks[2:]:
                    nc.vector.tensor_scalar(
                        yp[:, s_ : s_ + n_], ps_[:, :n_], a_sc[:], bconst[:],
                        op0=ALU.mult, op1=ALU.add,
                    )
                nc.vector.memset(yp[:, 0:MARG], 0.0)
                nc.vector.memset(yp[:, FLAT - 2 : FLAT], 0.0)
                fill_borders(yp)

            return finish

        def tap_idx(dh, dw):
            return (dh + 1) * 3 + (dw + 1)

        def tap_src_f(xpb, xpb2):
            def tap_src(dh, dw):
                t_off = WPAD * dh + dw
                if dw == 0:
                    s = OUT0 + t_off       # even
                    return xpb[:, s : s + OLEN]
                s = OUT0 - 1 + t_off       # even (t_off odd)
                return xpb2[:, s : s + OLEN]
            return tap_src

        # tile (0,1) depthwise is split: the DVE tree covers the first
        # DW_SPLIT flat elements; the PE (which would otherwise idle ~9us
        # waiting for the tree) computes the tail segments via diagonal
        # matmuls, exactly like tile (0,0).  2176 keeps the PE tail at the
        # same 4 banks (512 each -> 36 matmuls, no extra PE work vs 2560)
        # while cutting ~3us off the DVE tree: yp(0,1) gates big_conv(0)'s
        # ct1 pass and was landing ~2.4us after the PE ran dry.
        DW_SPLIT = 2176

        def produce_yp_early(b, ct, pre, post_stats_hook):
            """tile (0,1): latency-critical variant.  Taps use the RAW wsp
            weights (no stats dependency -- a_sc/bconst apply in one final
            rescale pass / fold into the PE-part PSUM drains), and TWO taps
            run on ACT.  post_stats_hook emits tile (0,0)'s deferred drains
            right after the shift copy so they don't delay the tree."""
            wsp, wpt, bis = pre["wsp"], pre["wpt"], pre["bis"]
            # diagonal weights for the PE part first: only needs wsp (early)
            diagw = small_pool.tile([128, 9, 128], bf16, tag="diagw")
            for t in range(9):
                nc.vector.tensor_scalar_mul(
                    diagw[:, t, :], ident[:], wsp[:, t : t + 1])
            xpb = xpb_pool.tile([128, FLAT], bf16, tag="xpb")
            xpb2 = xpb2_pool.tile([128, FLAT], bf16, tag="xpb2")
            nc.vector.memset(xpb[:, 0:MARG], 0.0)
            nc.vector.memset(xpb[:, FLAT - 2 : FLAT], 0.0)
            sumx = small_pool.tile([128, NQ], f32, tag="sumx")
            sumsq = small_pool.tile([128, 2], f32, tag="sumsq")
            # squares DEFERRED to after the shift copy: the tree's odd-tap
            # muls only need xpb2 (shift), and stats are only needed by the
            # final rescale pass -- this starts the DVE tree ~4us earlier.
            convert_passes(pre, xpb, sumx, dve_borders=False,
                           sumsq=None, xpb2=None)
            fill_borders(xpb)
            shift_copy(xpb, xpb2)

            if post_stats_hook is not None:
                post_stats_hook()
            square_passes(pre, sumsq, sq_junk)
            a_sc, bconst, _ = stats_finalize(sumx, sumsq, wsp, wpt, bis,
                                             scale_taps=False)

            yp = yp_pool.tile([128, FLAT], bf16, tag="yp")

            # ---- PE part: tail segments [OUT0+DW_SPLIT, OUT0+OLEN) ----
            pe_banks = []
            s = OUT0 + DW_SPLIT
            while s < OUT0 + OLEN:
                n = min(512, OUT0 + OLEN - s)
                ps = psum_pool.tile([128, 512], f32, tag="ps",
                                    name=f"psdw_{b}_{ct}_{s}")
                for t, (dh, dw) in enumerate(ALL_TAPS):
                    toff = WPAD * dh + dw
                    nc.tensor.matmul(
                        ps[:, :n], diagw[:, t, :], xpb[:, s + toff : s + toff + n],
                        start=(t == 0), stop=(t == 8),
                    )
                pe_banks.append((s, n, ps))
                s += n
            for s_, n_, ps_ in pe_banks:
                # DVE drain (see produce_yp_pe): ACT is saturated at b0
                nc.vector.tensor_scalar(
                    yp[:, s_ : s_ + n_], ps_[:, :n_], a_sc[:], bconst[:],
                    op0=ALU.mult, op1=ALU.add,
                )

            # ---- DVE tree part: [OUT0, OUT0+DW_SPLIT) ----
            L = DW_SPLIT
            yp_seg = yp[:, OUT0 : OUT0 + L]
            tap_src0 = tap_src_f(xpb, xpb2)

            def tap_src(dh, dw):
                return tap_src0(dh, dw)[:, :L]

            # ACT taps: the two even taps (0,0) and (1,0) (xpb-only reads)
            act_taps = [(0, 0), (1, 0)]
            tmpa = tmp_pool.tile([128, OLEN], bf16, tag="dwtmpa")
            nc.scalar.mul(tmpa[:, :L], tap_src(0, 0), wsp[:, tap_idx(0, 0) : tap_idx(0, 0) + 1])
            tmpd = tmp_pool.tile([128, OLEN], bf16, tag="dwtmpd", bufs=1, name=f"dwtmpd_{b}_{ct}")
            nc.scalar.mul(tmpd[:, :L], tap_src(1, 0), wsp[:, tap_idx(1, 0) : tap_idx(1, 0) + 1])

            d0 = (-1, 0)  # remaining even tap, on DVE, xpb-only
            rest = [t for t in ALL_TAPS if t not in act_taps and t != d0]  # 6 odd taps

            def mul_into(buf, tap):
                t = tap_idx(*tap)
                nc.vector.tensor_scalar_mul(buf[:, :L], tap_src(*tap), wsp[:, t : t + 1])

            ta = tmp_pool.tile([128, OLEN], bf16, tag="dwA", bufs=1, name=f"dwA_{b}_{ct}")
            tb = tmp_pool.tile([128, OLEN], bf16, tag="dwB", bufs=1, name=f"dwB_{b}_{ct}")
            tc_ = tmp_pool.tile([128, OLEN], bf16, tag="dwC", bufs=1, name=f"dwC_{b}_{ct}")
            mul_into(ta, rest[0])
            mul_into(tb, rest[1])
            nc.vector.tensor_add(ta[:, :L], ta[:, :L], tb[:, :L])
            mul_into(tb, rest[2])
            mul_into(tc_, rest[3])
            nc.vector.tensor_add(tb[:, :L], tb[:, :L], tc_[:, :L])
            nc.vector.tensor_add(ta[:, :L], ta[:, :L], tb[:, :L])   # 4 odd taps
            mul_into(tb, rest[4])
            mul_into(tc_, rest[5])
            nc.vector.tensor_add(tb[:, :L], tb[:, :L], tc_[:, :L])
            nc.vector.tensor_add(ta[:, :L], ta[:, :L], tb[:, :L])   # all 6 odd taps
            mul_into(tb, d0)
            nc.vector.tensor_add(tmpa[:, :L], tmpa[:, :L], tmpd[:, :L])  # ACT pair
            nc.vector.tensor_add(tb[:, :L], tb[:, :L], tmpa[:, :L])
            nc.vector.tensor_add(ta[:, :L], ta[:, :L], tb[:, :L])   # u = all 9 taps
            # final rescale (out-of-place): yp = u * a_sc + bconst
            nc.vector.tensor_scalar(
                yp_seg, ta[:, :L], a_sc[:], bconst[:], op0=ALU.mult, op1=ALU.add)

            nc.vector.memset(yp[:, 0:MARG], 0.0)
            nc.vector.memset(yp[:, FLAT - 2 : FLAT], 0.0)
            fill_borders(yp)
            yp_tiles[(b, ct)] = yp

        def produce_yp_steady(b, ct, pre, act2=False):
            """norm + depthwise pipeline, steady-state variant.  act2: run
            TWO taps on ACT instead of one -- used for the ct=1 tile whose
            completion gates the big-conv ct1 pass ~15.5us into each batch
            (DVE is the tighter engine there; ACT has slack)."""
            wsp, wpt, bis = pre["wsp"], pre["wpt"], pre["bis"]
            xpb = xpb_pool.tile([128, FLAT], bf16, tag="xpb")
            xpb2 = xpb2_pool.tile([128, FLAT], bf16, tag="xpb2")
            nc.vector.memset(xpb[:, 0:MARG], 0.0)
            nc.vector.memset(xpb[:, FLAT - 2 : FLAT], 0.0)
            sumx = small_pool.tile([128, NQ], f32, tag="sumx")
            sumsq = small_pool.tile([128, 2], f32, tag="sumsq")
            convert_passes(pre, xpb, sumx, dve_borders=False,
                           sumsq=sumsq, xpb2=xpb2)
            fill_borders(xpb)

            # shifted copy (one element left) for 4B-aligned odd-offset
            # taps (DVE bf16 2x mode). NOTE: gpsimd bulk ops are poison here
            # -- they hold the shared DVE/GpSimd SBUF port for their whole
            # duration and stall every DVE tensor_tensor op; stays on ACT.
            shift_copy(xpb, xpb2)

            _, bconst, wsc = stats_finalize(sumx, sumsq, wsp, wpt, bis,
                                            scale_taps=True)

            yp = yp_pool.tile([128, FLAT], bf16, tag="yp")
            yp_seg = yp[:, OUT0 : OUT0 + OLEN]
            tap_src = tap_src_f(xpb, xpb2)

            # center tap's multiply runs on ACT (it has slack); the other
            # taps' products come from DVE 2x-mode tensor_scalar muls, then
            # are combined with a pairwise ADD TREE (same op count as a
            # serial chain but 4x shorter dependency depth and ~2x better
            # bf16 rounding error).  ACT muls are split in halves so they
            # never block the ACT queue for >2us.
            def act_mul(buf, tap):
                t = tap_idx(*tap)
                h = OLEN // 2
                nc.scalar.mul(buf[:, :h], tap_src(*tap)[:, :h],
                              wsc[:, t : t + 1])
                nc.scalar.mul(buf[:, h:], tap_src(*tap)[:, h:],
                              wsc[:, t : t + 1])

            tmpa = tmp_pool.tile([128, OLEN], bf16, tag="dwtmpa")
            act_mul(tmpa, (0, 0))

            # tap0 writes yp_seg = w0*x0 + B directly
            d0, w0 = ALL_TAPS[0]
            t0 = tap_idx(d0, w0)
            nc.vector.tensor_scalar(
                yp_seg, tap_src(d0, w0), wsc[:, t0 : t0 + 1], bconst[:],
                op0=ALU.mult, op1=ALU.add,
            )

            def mul_into(buf, tap):
                t = tap_idx(*tap)
                nc.vector.tensor_scalar_mul(buf[:], tap_src(*tap), wsc[:, t : t + 1])

            ta = tmp_pool.tile([128, OLEN], bf16, tag="dwA", bufs=1, name=f"dwA_{b}_{ct}")
            tb = tmp_pool.tile([128, OLEN], bf16, tag="dwB", bufs=1, name=f"dwB_{b}_{ct}")
            tc_ = tmp_pool.tile([128, OLEN], bf16, tag="dwC", bufs=1, name=f"dwC_{b}_{ct}")
            if act2:
                # second ACT tap (1,0): DVE drops to 6 muls + 8 adds
                tmpd = tmp_pool.tile([128, OLEN], bf16, tag="dwtmpd",
                                     bufs=1, name=f"dwtmpd_{b}_{ct}")
                act_mul(tmpd, (1, 0))
                rest = [t for t in ALL_TAPS[1:] if t not in ((0, 0), (1, 0))]
                mul_into(ta, rest[0])
                mul_into(tb, rest[1])
                nc.vector.tensor_add(ta[:], ta[:], tb[:])
                mul_into(tb, rest[2])
                mul_into(tc_, rest[3])
                nc.vector.tensor_add(tb[:], tb[:], tc_[:])
                nc.vector.tensor_add(ta[:], ta[:], tb[:])    # 4 DVE taps
                mul_into(tb, rest[4])
                mul_into(tc_, rest[5])
                nc.vector.tensor_add(tb[:], tb[:], tc_[:])
                nc.vector.tensor_add(tc_[:], tmpa[:], tmpd[:])  # ACT pair
                nc.vector.tensor_add(tb[:], tb[:], tc_[:])
                nc.vector.tensor_add(yp_seg, yp_seg, ta[:])
                nc.vector.tensor_add(yp_seg, yp_seg, tb[:])
            else:
                rest = [t for t in ALL_TAPS[1:] if t != (0, 0)]  # 7 taps
                mul_into(ta, rest[0])
                mul_into(tb, rest[1])
                nc.vector.tensor_add(ta[:], ta[:], tb[:])
                mul_into(tb, rest[2])
                mul_into(tc_, rest[3])
                nc.vector.tensor_add(tb[:], tb[:], tc_[:])
                nc.vector.tensor_add(ta[:], ta[:], tb[:])        # taps 1-4
                mul_into(tb, rest[4])
                mul_into(tc_, rest[5])
                nc.vector.tensor_add(tb[:], tb[:], tc_[:])
                mul_into(tc_, rest[6])
                nc.vector.tensor_add(tc_[:], tc_[:], tmpa[:])    # + ACT tap
                nc.vector.tensor_add(tb[:], tb[:], tc_[:])       # taps 5-7 + act
                nc.vector.tensor_add(yp_seg, yp_seg, ta[:])
                nc.vector.tensor_add(yp_seg, yp_seg, tb[:])

            nc.vector.memset(yp[:, 0:MARG], 0.0)
            nc.vector.memset(yp[:, FLAT - 2 : FLAT], 0.0)
            fill_borders(yp)
            yp_tiles[(b, ct)] = yp

        # tap order for the ct-outer paths: center tap first -- its rhs
        # reads only interior yp cells, so the bank-start matmuls don't
        # wait on the border-fill ACT ops.
        CT_OUTER_TAPS = [(0, 0)] + [
            (dh, dw) for dh in (-1, 0, 1) for dw in (-1, 0, 1) if (dh, dw) != (0, 0)
        ]

        def drain_bank(b, ot, stage, r0, nr, p, out_slice):
            src = p[:].rearrange("p (r c) -> p r c", c=W)
            nc.scalar.activation(
                out=stage[:, r0 : r0 + nr, :], in_=src,
                func=AF.Identity, bias=cb_sb[:, ot : ot + 1],
            )
            rend = r0 + nr
            if rend % out_slice == 0:
                s0 = rend - out_slice
                nc.sync.dma_start(
                    out=out_ext[b, ot * 128 : (ot + 1) * 128, s0:rend],
                    in_=stage[:, s0:rend],
                )

        def big_conv(b):
            """Structure (per ot): for ot0 of batches>0, the ct0 taps run
            tap-outer first, giving PE ~15.5us of runway on yp[b,0] alone
            while DVE finishes yp[b,1].  Everything else runs BANK-OUTER
            with an immediate per-bank drain, so PSUM banks recycle
            continuously -- the old all-8-banks-finish-at-once shape made
            every ot/batch transition stall ~2-3us on queued ACT drains."""
            last = b == nb - 1
            for ot in range(OT):
                stage = stage_pool.tile([128, H, W], bf16, tag="stage")
                # 8-row slices for the final ot so the very last out DMA
                # (which the kernel-end barrier waits on) is small
                out_slice = 8 if (last and ot == OT - 1) else OUT_SLICE
                ps = {}
                for r0, nr in ROW_BLOCKS:
                    ps[r0] = psum_pool.tile(
                        [128, BLK_ROWS * W], f32, tag="ps",
                        name=f"ps_{b}_{ot}_{r0}",
                    )
                runway = ot == 0 and b > 0
                if runway:
                    ypg = grid(yp_tiles[(b, 0)][:])
                    for ti, (dh, dw) in enumerate(CT_OUTER_TAPS):
                        lhsT = cw_sb[0][:, dh + 1, dw + 1, ot, :]
                        for r0, nr in ROW_BLOCKS:
                            rhs = ypg[:, r0 + 1 + dh : r0 + 1 + dh + nr,
                                      1 + dw : 1 + dw + W]
                            nc.tensor.matmul(ps[r0][:], lhsT, rhs,
                                             start=(ti == 0), stop=False)
                rest_cts = [1] if runway else list(range(CT))
                n_acc = len(rest_cts) * 9
                for r0, nr in ROW_BLOCKS:
                    i = 0
                    for ct in rest_cts:
                        ypg = grid(yp_tiles[(b, ct)][:])
                        for dh, dw in CT_OUTER_TAPS:
                            lhsT = cw_sb[ct][:, dh + 1, dw + 1, ot, :]
                            rhs = ypg[:, r0 + 1 + dh : r0 + 1 + dh + nr,
                                      1 + dw : 1 + dw + W]
                            nc.tensor.matmul(
                                ps[r0][:], lhsT, rhs,
                                start=(not runway and i == 0),
                                stop=(i == n_acc - 1),
                            )
                            i += 1
                    drain_bank(b, ot, stage, r0, nr, ps[r0], out_slice)

        # Emission order doubles as DMA-issue order (Sync engine serializes
        # issues at ~0.65us each): batch 0's x slices go absolutely first,
        # then the weights, then each later batch's x prefetches interleave
        # ahead of the previous batch's big_conv.
        pe_warmup()
        pre = {}
        pre[(0, 0)] = prefetch_x(0, 0, wsp_early=True)
        pre[(0, 1)] = prefetch_x(0, 1)
        dw_finish = produce_yp_pe(0, 0, pre[(0, 0)])
        load_cw()
        produce_yp_early(0, 1, pre[(0, 1)], post_stats_hook=dw_finish)
        for b in range(nb):
            if b + 1 < nb:
                pre[(b + 1, 0)] = prefetch_x(b + 1, 0)
                pre[(b + 1, 1)] = prefetch_x(b + 1, 1)
            big_conv(b)
            if b + 1 < nb:
                produce_yp_steady(b + 1, 0, pre[(b + 1, 0)])
                produce_yp_steady(b + 1, 1, pre[(b + 1, 1)], act2=True)

    nc.compile()
    return nc


def _host_prep(x, w_spatial, w_pointwise, bias, conv_w, conv_b, nb=NB):
    import ml_dtypes

    ncores = x.shape[0] // nb
    cw = np.ascontiguousarray(
        conv_w.reshape(OT, 128, CT, 128, 3, 3).transpose(2, 3, 4, 5, 0, 1)
    ).astype(ml_dtypes.bfloat16)
    cb = np.ascontiguousarray(conv_b.reshape(OT, 128)).astype(np.float32)
    wsp = np.ascontiguousarray(w_spatial.reshape(-1, CT, 128, 9)).astype(np.float32)
    wpt = np.ascontiguousarray(w_pointwise.reshape(-1, CT, 128)).astype(np.float32)
    bis = np.ascontiguousarray(bias.reshape(-1, CT, 128)).astype(np.float32)
    x = np.ascontiguousarray(x).astype(ml_dtypes.bfloat16)
    in_maps = []
    for i in range(ncores):
        sl = slice(i * nb, (i + 1) * nb)
        in_maps.append({
            "x": np.ascontiguousarray(x[sl]),
            "wsp": np.ascontiguousarray(wsp[sl]),
            "wpt": np.ascontiguousarray(wpt[sl]),
            "bis": np.ascontiguousarray(bis[sl]),
            "cw": cw,
            "cb": cb,
        })
    return in_maps


def _run(inputs, trace=False):
    from concourse.bass_utils import run_bass_kernel_spmd

    if "nc" not in _CACHED:
        _CACHED["nc"] = _build()
    nc = _CACHED["nc"]
    in_maps = _host_prep(**inputs)
    kw = {}
    if trace:
        import shutil
        tdir = "/tmp/kernel_trace_out"
        shutil.rmtree(tdir, ignore_errors=True)
        os.makedirs(tdir, exist_ok=True)
        kw["tmpdir"] = tdir
    res = run_bass_kernel_spmd(
        nc, in_maps, core_ids=list(range(N_CORES)), trace=trace, **kw
    )
    out = np.concatenate([res.results[i]["out"] for i in range(N_CORES)], axis=0)
    return out.astype(np.float32), res


def kernel(x, w_spatial, w_pointwise, bias, conv_w, conv_b):
    out, _ = _run(
        dict(x=np.asarray(x), w_spatial=np.asarray(w_spatial),
             w_pointwise=np.asarray(w_pointwise), bias=np.asarray(bias),
             conv_w=np.asarray(conv_w), conv_b=np.asarray(conv_b)),
        trace=bool(int(os.environ.get("KERNEL_TRACE", "0"))),
    )
    return out



# revision 66
# speedup vs baseline: 1.1976x; 1.0025x over previous
"""AdaConv2d fused kernel for 8 TRN2 NeuronCores (pure data parallel).

Per-sample pipeline (all fused on-chip):
  1. instance-norm stats (mean/var over HW)
  2. dynamic per-(b,c) depthwise 3x3 conv with reflect padding
  3. per-(b,c) scale+bias (folded algebraically into the depthwise taps:
     y = A*(sum_t w_t * x_t) + B with A = rstd*w_pt, B = bias - mu*A*sum(w))
  4. fixed 3x3 conv (256->256) with reflect padding, as 18 accumulated
     bf16 matmuls per PSUM block

Layout: channels on partitions (2 tiles of 128), pixels on the free axis.
Padded images are 66 rows x 66 cols stored flat with a 2-element leading
margin (so every depthwise tap and every matmul rhs is a fully CONTIGUOUS
1D slice).  flat(r, c) = 2 + 66*r + c.  Rows 0/65 and cols 0/65 are the
reflect pads.  A one-element-left-shifted copy (xpb2[i] = xpb[i+1]) keeps
all odd-offset depthwise taps 4-byte aligned for the DVE bf16 2x mode.

Perf notes (measured on hw):
  - bf16 matmul N=512 paces at ~216 ns warm; 3D-AP rhs costs ~nothing.
    Slow stretches in traces are PE p-state/DVFS ramp, not AP shape.
  - DVE: TS muls ~2x (1.25us/4224), TT adds 1x (2.35us), STT and custom
    DVE ops 0.5x (4.6us) -> the TS-mul + TT-add-tree depthwise is optimal.
  - fp8 DoubleRow matmul paces 427ns for K=256 = zero gain over 2x bf16.
  - First-batch critical path: x DMA in 4 slices, ACT converts first,
    PE depthwise uses UNSCALED taps (stats fold into the PSUM drain) so
    the first matmul doesn't wait for stats.
"""

import os
from contextlib import ExitStack

import numpy as np

B_GLOBAL = 32
N_CORES = 8
NB = B_GLOBAL // N_CORES  # batches per core
C = 256
H = W = 64
WPAD = W + 2        # 66 padded row length
HPAD = H + 2        # 66 padded rows
MARG = 2            # leading margin so tap windows stay in-bounds
FLAT = MARG + HPAD * WPAD + 2   # 4360 flat elements per padded image
NPIX = H * W        # 4096
CT = C // 128       # channel tiles
OT = C // 128       # out-channel tiles
EPS = 1e-5
BLK_ROWS = 8        # output rows per PSUM block (8*64=512 fp32, 3D-AP rhs)

ROW_BLOCKS = [(r0, BLK_ROWS) for r0 in range(0, H, BLK_ROWS)]
OUT_SLICE = 16      # rows per output DMA slice (tail-latency hiding)

_CACHED = {}


def _build(nb=NB):
    import concourse.mybir as mybir
    import concourse.tile as tile
    from concourse import bacc

    f32 = mybir.dt.float32
    bf16 = mybir.dt.bfloat16
    AF = mybir.ActivationFunctionType
    ALU = mybir.AluOpType

    nc = bacc.Bacc(None, target_bir_lowering=False)

    # x and out travel as bf16 (host converts): halves both big DMA streams;
    # measured end-to-end numeric impact ~3.2e-3 rel err (vs 2e-2 budget).
    x_ext = nc.declare_dram_parameter("x", [nb, C, H, W], bf16, isOutput=False)
    wsp_ext = nc.declare_dram_parameter("wsp", [nb, CT, 128, 9], f32, isOutput=False)
    wpt_ext = nc.declare_dram_parameter("wpt", [nb, CT, 128], f32, isOutput=False)
    bis_ext = nc.declare_dram_parameter("bis", [nb, CT, 128], f32, isOutput=False)
    cw_ext = nc.declare_dram_parameter("cw", [CT, 128, 3, 3, OT, 128], bf16, isOutput=False)
    cb_ext = nc.declare_dram_parameter("cb", [OT, 128], f32, isOutput=False)
    out_ext = nc.declare_dram_parameter("out", [nb, C, H, W], bf16, isOutput=True)

    with tile.TileContext(nc) as tc, ExitStack() as ctx:
        singles = ctx.enter_context(tc.tile_pool(name="singles", bufs=1))
        xin_pool = ctx.enter_context(tc.tile_pool(name="xin", bufs=4))
        xpb_pool = ctx.enter_context(tc.tile_pool(name="xpb", bufs=2))
        xpb2_pool = ctx.enter_context(tc.tile_pool(name="xpb2", bufs=2))
        yp_pool = ctx.enter_context(tc.tile_pool(name="yp", bufs=4))
        stage_pool = ctx.enter_context(tc.tile_pool(name="stage", bufs=2))
        small_pool = ctx.enter_context(tc.tile_pool(name="small", bufs=6))
        tmp_pool = ctx.enter_context(tc.tile_pool(name="tmp", bufs=2))
        psum_pool = ctx.enter_context(tc.tile_pool(name="psum", bufs=8, space="PSUM"))

        # ---- constants / fixed weights ----
        # (emitted lazily AFTER batch 0's x DMAs so the 1.2MB weight load
        # doesn't delay the first tile's data; weights are only needed at
        # ~40us when the first big-conv matmul fires)
        cw_sb = []

        cb_sb = singles.tile([128, OT], f32, tag="cb")

        def load_cw():
            for ct in range(CT):
                t = singles.tile([128, 3, 3, OT, 128], bf16, tag=f"cw{ct}")
                nc.sync.dma_start(out=t[:], in_=cw_ext[ct])
                cw_sb.append(t)
            for ot in range(OT):
                nc.sync.dma_start(out=cb_sb[:, ot : ot + 1], in_=cb_ext[ot, :, None])
        eps_sb = singles.tile([128, 1], f32, tag="eps")
        nc.vector.memset(eps_sb[:], EPS)
        ident = singles.tile([128, 128], bf16, tag="ident")
        from concourse.masks import make_identity
        make_identity(nc, ident[:])
        # touch the Sqrt activation table once so its ~1.3us load happens
        # before the first tile's stats need it
        warm = singles.tile([128, 1], f32, tag="warm")
        nc.scalar.activation(out=warm[:], in_=eps_sb[:], func=AF.Sqrt, bias=eps_sb[:])

        yp_tiles = {}

        def grid(flat_ap):
            """(128, FLAT) flat padded buffer -> (128, 66, 66) image view."""
            return flat_ap[:, MARG : MARG + HPAD * WPAD].rearrange(
                "p (r c) -> p r c", c=WPAD)

        def fill_borders(buf):
            g = grid(buf[:])
            nc.scalar.copy(out=g[:, 1 : 1 + H, 0:1], in_=g[:, 1 : 1 + H, 2:3])
            nc.scalar.copy(out=g[:, 1 : 1 + H, 65:66], in_=g[:, 1 : 1 + H, 63:64])
            nc.scalar.copy(out=g[:, 0], in_=g[:, 2])
            nc.scalar.copy(out=g[:, HPAD - 1], in_=g[:, HPAD - 3])

        def stats_finalize(sumx, sumsq, wsp, wpt, bis, scale_taps):
            """Common stats tail: returns (a_sc, bconst, wsc-or-None)."""
            sx = small_pool.tile([128, 1], f32, tag="sx")
            nc.vector.reduce_sum(sx[:], sumx[:], axis=mybir.AxisListType.X)
            sq = small_pool.tile([128, 1], f32, tag="sq")
            nc.vector.reduce_sum(sq[:], sumsq[:], axis=mybir.AxisListType.X)
            mu = small_pool.tile([128, 1], f32, tag="mu")
            nc.vector.tensor_scalar_mul(mu[:], sx[:], 1.0 / NPIX)
            m2 = small_pool.tile([128, 1], f32, tag="m2")
            nc.vector.tensor_scalar_mul(m2[:], sq[:], 1.0 / NPIX)
            musq = small_pool.tile([128, 1], f32, tag="musq")
            nc.vector.tensor_mul(musq[:], mu[:], mu[:])
            var = small_pool.tile([128, 1], f32, tag="var")
            nc.vector.tensor_sub(var[:], m2[:], musq[:])
            std = small_pool.tile([128, 1], f32, tag="std")
            nc.scalar.activation(out=std[:], in_=var[:], func=AF.Sqrt, bias=eps_sb[:])
            rstd = small_pool.tile([128, 1], f32, tag="rstd")
            nc.vector.reciprocal(out=rstd[:], in_=std[:])
            a_sc = small_pool.tile([128, 1], f32, tag="a_sc")
            nc.vector.tensor_mul(a_sc[:], rstd[:], wpt[:])
            sw = small_pool.tile([128, 1], f32, tag="sw")
            nc.vector.reduce_sum(sw[:], wsp[:], axis=mybir.AxisListType.X)
            t1 = small_pool.tile([128, 1], f32, tag="t1")
            nc.vector.tensor_mul(t1[:], mu[:], a_sc[:])
            nc.vector.tensor_mul(t1[:], t1[:], sw[:])
            bconst = small_pool.tile([128, 1], f32, tag="bconst")
            nc.vector.tensor_sub(bconst[:], bis[:], t1[:])
            if not scale_taps:
                return a_sc, bconst, None
            wsc = small_pool.tile([128, 9], f32, tag="wsc")
            nc.vector.tensor_scalar_mul(wsc[:], wsp[:], a_sc[:])
            return a_sc, bconst, wsc

        NQ = 4          # x DMA slices per tile
        QROWS = H // NQ

        # shared junk target for the ACT Square passes of tiles that have
        # no xpb2 buffer of their own (the PE tile)
        sq_junk = singles.tile([128, NPIX], bf16, tag="sqjunk")

        OUT0, OLEN = 68, 4224
        ALL_TAPS = [(dh, dw) for dh in (-1, 0, 1) for dw in (-1, 0, 1)]

        def pe_warmup(n_mm=14):
            """Junk matmuls at t~6us: the PE HAM clock-gate needs ~3.4us of
            sustained busy to lift K=4/8 -> 8/8.  Running these during the
            (otherwise PE-idle) initial x DMA window means the real depthwise
            matmuls start at full clock instead of paying ~2x for 5us."""
            wjunk = singles.tile([128, 512], bf16, tag="wjunk")
            nc.vector.memset(wjunk[:], 0.0)
            wps = psum_pool.tile([128, 512], f32, tag="ps", name="ps_warm")
            for _ in range(n_mm):
                nc.tensor.matmul(wps[:], wjunk[:, 0:128], wjunk[:],
                                 start=True, stop=True)

        def prefetch_x(b, ct, wsp_early=False):
            """Issue the x-slice + per-tile weight DMAs as early as possible.
            DMA issues serialize at ~0.65us each on the Sync engine, so
            emission order here IS the issue order that decides when each
            tile's data lands.  wsp_early (batch-0 tile 0): wsp right after
            the first x slice so diagw can be built before the x converts
            finish -- the first depthwise matmul needs both."""
            xf = xin_pool.tile([128, H, W], bf16, tag="xf")
            wsp = small_pool.tile([128, 9], f32, tag="wsp")
            for q in range(NQ):
                nc.sync.dma_start(
                    out=xf[:, q * QROWS : (q + 1) * QROWS],
                    in_=x_ext[b, ct * 128 : (ct + 1) * 128, q * QROWS : (q + 1) * QROWS],
                )
                if q == 0 and wsp_early:
                    nc.sync.dma_start(out=wsp[:], in_=wsp_ext[b, ct])
            if not wsp_early:
                nc.sync.dma_start(out=wsp[:], in_=wsp_ext[b, ct])
            wpt = small_pool.tile([128, 1], f32, tag="wpt")
            nc.sync.dma_start(out=wpt[:], in_=wpt_ext[b, ct, :, None])
            bis = small_pool.tile([128, 1], f32, tag="bis")
            nc.sync.dma_start(out=bis[:], in_=bis_ext[b, ct, :, None])
            return {"xf": xf, "wsp": wsp, "wpt": wpt, "bis": bis}

        def shift_copy(xpb, xpb2, chunks=3):
            """xpb2[i] = xpb[i+1] on ACT, split into chunks: a single 4358-el
            pass is ~3.9us of ACT head-of-line blocking for whatever gets
            scheduled behind it (border fills, drains); ~1.3us chunks bound
            that."""
            n = FLAT - 2
            step = -(-n // chunks)
            s = 0
            while s < n:
                e = min(n, s + step)
                nc.scalar.copy(out=xpb2[:, s:e], in_=xpb[:, s + 1 : e + 1])
                s = e
            nc.vector.memset(xpb2[:, FLAT - 2 : FLAT], 0.0)

        def square_passes(pre, sumsq, junk):
            """sum(x^2) via ACT Square; junk output target."""
            xff = pre["xf"][:].rearrange("p a b -> p (a b)")
            for hh in range(2):
                lo = hh * (NPIX // 2)
                hi = lo + NPIX // 2
                nc.scalar.activation(
                    out=junk[:, lo:hi], in_=xff[:, lo:hi],
                    func=AF.Square, accum_out=sumsq[:, hh : hh + 1],
                )

        def convert_passes(pre, xpb, sumx, dve_borders, sumsq, xpb2):
            """grid placement + sum(x) on ACT; sum(x^2) after unless
            sumsq is None (caller emits square_passes itself, later)."""
            xf = pre["xf"]
            for q in range(NQ):
                nc.scalar.activation(
                    out=grid(xpb[:])[:, 1 + q * QROWS : 1 + (q + 1) * QROWS, 1 : 1 + W],
                    in_=xf[:, q * QROWS : (q + 1) * QROWS],
                    func=AF.Copy, accum_out=sumx[:, q : q + 1],
                )
                if dve_borders:
                    g = grid(xpb[:])
                    r0 = 1 + q * QROWS
                    nc.vector.tensor_copy(out=g[:, r0 : r0 + QROWS, 0:1],
                                          in_=g[:, r0 : r0 + QROWS, 2:3])
                    nc.vector.tensor_copy(out=g[:, r0 : r0 + QROWS, 65:66],
                                          in_=g[:, r0 : r0 + QROWS, 63:64])
                    if q == 0:
                        nc.vector.tensor_copy(out=g[:, 0], in_=g[:, 2])
                    if q == NQ - 1:
                        nc.vector.tensor_copy(out=g[:, HPAD - 1], in_=g[:, HPAD - 3])
            if sumsq is not None:
                square_passes(pre, sumsq, xpb2)

        def produce_yp_pe(b, ct, pre):
            """tile (0,0): depthwise on the (still idle) PE via diagonal
            matmuls with UNSCALED taps (no stats dependency); a_sc/bconst
            fold into the PSUM drains.  Drains for the first 2 banks run on
            DVE (idle) so the PSUM pool never stalls the PE; the rest are
            deferred (returned as a finisher) behind tile (0,1)'s ACT ops."""
            wsp, wpt, bis = pre["wsp"], pre["wpt"], pre["bis"]
            xpb = xpb_pool.tile([128, FLAT], bf16, tag="xpb")
            nc.vector.memset(xpb[:, 0:MARG], 0.0)
            nc.vector.memset(xpb[:, FLAT - 2 : FLAT], 0.0)
            sumx = small_pool.tile([128, NQ], f32, tag="sumx")
            sumsq = small_pool.tile([128, 2], f32, tag="sumsq")
            convert_passes(pre, xpb, sumx, dve_borders=True,
                           sumsq=sumsq, xpb2=sq_junk)

            yp = yp_pool.tile([128, FLAT], bf16, tag="yp")
            diagw = small_pool.tile([128, 9, 128], bf16, tag="diagw")
            for t in range(9):
                nc.vector.tensor_scalar_mul(
                    diagw[:, t, :], ident[:], wsp[:, t : t + 1])
            a_sc, bconst, _ = stats_finalize(sumx, sumsq, wsp, wpt, bis,
                                             scale_taps=False)
            banks = []
            s = OUT0
            while s < OUT0 + OLEN:
                n = min(512, OUT0 + OLEN - s)
                ps = psum_pool.tile([128, 512], f32, tag="ps",
                                    name=f"psdw_{b}_{ct}_{s}")
                for t, (dh, dw) in enumerate(ALL_TAPS):
                    toff = WPAD * dh + dw
                    nc.tensor.matmul(
                        ps[:, :n], diagw[:, t, :], xpb[:, s + toff : s + toff + n],
                        start=(t == 0), stop=(t == 8),
                    )
                banks.append((s, n, ps))
                if len(banks) <= 2:
                    # first two drains emitted inline (the scheduler runs
                    # them early): frees PSUM slot 0/1 before the 9th bank
                    # allocation needs one.  dw(0,0) drains run on DVE --
                    # their stats dep clears at ~20us, before the (0,1)
                    # tree dominates the DVE queue, and keeping them off
                    # ACT avoids pushing batch-1's converts later (which
                    # showed up as a 7.5us b0->b1 stall).
                    s_, n_, ps_ = banks[-1]
                    nc.vector.tensor_scalar(
                        yp[:, s_ : s_ + n_], ps_[:, :n_], a_sc[:], bconst[:],
                        op0=ALU.mult, op1=ALU.add,
                    )
                s += n
            yp_tiles[(b, ct)] = yp

            def finish():
                for s_, n_, ps_ in banks[2:]:
                    nc.vector.tensor_scalar(
                        yp[:, s_ : s_ + n_], ps_[:, :n_], a_sc[:], bconst[:],
                        op0=ALU.mult, op1=ALU.add,
                    )
                nc.vector.memset(yp[:, 0:MARG], 0.0)
                nc.vector.memset(yp[:, FLAT - 2 : FLAT], 0.0)
                fill_borders(yp)

            return finish

        def tap_idx(dh, dw):
            return (dh + 1) * 3 + (dw + 1)

        def tap_src_f(xpb, xpb2):
            def tap_src(dh, dw):
                t_off = WPAD * dh + dw
                if dw == 0:
                    s = OUT0 + t_off       # even
                    return xpb[:, s : s + OLEN]
                s = OUT0 - 1 + t_off       # even (t_off odd)
                return xpb2[:, s : s + OLEN]
            return tap_src

        # tile (0,1) depthwise is split: the DVE tree covers the first
        # DW_SPLIT flat elements; the PE (which would otherwise idle ~9us
        # waiting for the tree) computes the tail segments via diagonal
        # matmuls, exactly like tile (0,0).  2176 keeps the PE tail at the
        # same 4 banks (512 each -> 36 matmuls, no extra PE work vs 2560)
        # while cutting ~3us off the DVE tree: yp(0,1) gates big_conv(0)'s
        # ct1 pass and was landing ~2.4us after the PE ran dry.
        DW_SPLIT = 2176

        def produce_yp_early(b, ct, pre, post_stats_hook):
            """tile (0,1): latency-critical variant.  Taps use the RAW wsp
            weights (no stats dependency -- a_sc/bconst apply in one final
            rescale pass / fold into the PE-part PSUM drains), and TWO taps
            run on ACT.  post_stats_hook emits tile (0,0)'s deferred drains
            right after the shift copy so they don't delay the tree."""
            wsp, wpt, bis = pre["wsp"], pre["wpt"], pre["bis"]
            # diagonal weights for the PE part first: only needs wsp (early)
            diagw = small_pool.tile([128, 9, 128], bf16, tag="diagw")
            for t in range(9):
                nc.vector.tensor_scalar_mul(
                    diagw[:, t, :], ident[:], wsp[:, t : t + 1])
            xpb = xpb_pool.tile([128, FLAT], bf16, tag="xpb")
            xpb2 = xpb2_pool.tile([128, FLAT], bf16, tag="xpb2")
            nc.vector.memset(xpb[:, 0:MARG], 0.0)
            nc.vector.memset(xpb[:, FLAT - 2 : FLAT], 0.0)
            sumx = small_pool.tile([128, NQ], f32, tag="sumx")
            sumsq = small_pool.tile([128, 2], f32, tag="sumsq")
            # squares DEFERRED to after the shift copy: the tree's odd-tap
            # muls only need xpb2 (shift), and stats are only needed by the
            # final rescale pass -- this starts the DVE tree ~4us earlier.
            convert_passes(pre, xpb, sumx, dve_borders=False,
                           sumsq=None, xpb2=None)
            fill_borders(xpb)
            shift_copy(xpb, xpb2)

            if post_stats_hook is not None:
                post_stats_hook()
            square_passes(pre, sumsq, sq_junk)
            a_sc, bconst, _ = stats_finalize(sumx, sumsq, wsp, wpt, bis,
                                             scale_taps=False)

            yp = yp_pool.tile([128, FLAT], bf16, tag="yp")

            # ---- PE part: tail segments [OUT0+DW_SPLIT, OUT0+OLEN) ----
            pe_banks = []
            s = OUT0 + DW_SPLIT
            while s < OUT0 + OLEN:
                n = min(512, OUT0 + OLEN - s)
                ps = psum_pool.tile([128, 512], f32, tag="ps",
                                    name=f"psdw_{b}_{ct}_{s}")
                for t, (dh, dw) in enumerate(ALL_TAPS):
                    toff = WPAD * dh + dw
                    nc.tensor.matmul(
                        ps[:, :n], diagw[:, t, :], xpb[:, s + toff : s + toff + n],
                        start=(t == 0), stop=(t == 8),
                    )
                pe_banks.append((s, n, ps))
                s += n
            for s_, n_, ps_ in pe_banks:
                # DVE drain (see produce_yp_pe): ACT is saturated at b0
                nc.vector.tensor_scalar(
                    yp[:, s_ : s_ + n_], ps_[:, :n_], a_sc[:], bconst[:],
                    op0=ALU.mult, op1=ALU.add,
                )

            # ---- DVE tree part: [OUT0, OUT0+DW_SPLIT) ----
            L = DW_SPLIT
            yp_seg = yp[:, OUT0 : OUT0 + L]
            tap_src0 = tap_src_f(xpb, xpb2)

            def tap_src(dh, dw):
                return tap_src0(dh, dw)[:, :L]

            # ACT taps: the two even taps (0,0) and (1,0) (xpb-only reads)
            act_taps = [(0, 0), (1, 0)]
            tmpa = tmp_pool.tile([128, OLEN], bf16, tag="dwtmpa")
            nc.scalar.mul(tmpa[:, :L], tap_src(0, 0), wsp[:, tap_idx(0, 0) : tap_idx(0, 0) + 1])
            tmpd = tmp_pool.tile([128, OLEN], bf16, tag="dwtmpd", bufs=1, name=f"dwtmpd_{b}_{ct}")
            nc.scalar.mul(tmpd[:, :L], tap_src(1, 0), wsp[:, tap_idx(1, 0) : tap_idx(1, 0) + 1])

            d0 = (-1, 0)  # remaining even tap, on DVE, xpb-only
            rest = [t for t in ALL_TAPS if t not in act_taps and t != d0]  # 6 odd taps

            def mul_into(buf, tap):
                t = tap_idx(*tap)
                nc.vector.tensor_scalar_mul(buf[:, :L], tap_src(*tap), wsp[:, t : t + 1])

            ta = tmp_pool.tile([128, OLEN], bf16, tag="dwA", bufs=1, name=f"dwA_{b}_{ct}")
            tb = tmp_pool.tile([128, OLEN], bf16, tag="dwB", bufs=1, name=f"dwB_{b}_{ct}")
            tc_ = tmp_pool.tile([128, OLEN], bf16, tag="dwC", bufs=1, name=f"dwC_{b}_{ct}")
            mul_into(ta, rest[0])
            mul_into(tb, rest[1])
            nc.vector.tensor_add(ta[:, :L], ta[:, :L], tb[:, :L])
            mul_into(tb, rest[2])
            mul_into(tc_, rest[3])
            nc.vector.tensor_add(tb[:, :L], tb[:, :L], tc_[:, :L])
            nc.vector.tensor_add(ta[:, :L], ta[:, :L], tb[:, :L])   # 4 odd taps
            mul_into(tb, rest[4])
            mul_into(tc_, rest[5])
            nc.vector.tensor_add(tb[:, :L], tb[:, :L], tc_[:, :L])
            nc.vector.tensor_add(ta[:, :L], ta[:, :L], tb[:, :L])   # all 6 odd taps
            mul_into(tb, d0)
            nc.vector.tensor_add(tmpa[:, :L], tmpa[:, :L], tmpd[:, :L])  # ACT pair
            nc.vector.tensor_add(tb[:, :L], tb[:, :L], tmpa[:, :L])
            nc.vector.tensor_add(ta[:, :L], ta[:, :L], tb[:, :L])   # u = all 9 taps
            # final rescale (out-of-place): yp = u * a_sc + bconst
            nc.vector.tensor_scalar(
                yp_seg, ta[:, :L], a_sc[:], bconst[:], op0=ALU.mult, op1=ALU.add)

            nc.vector.memset(yp[:, 0:MARG], 0.0)
            nc.vector.memset(yp[:, FLAT - 2 : FLAT], 0.0)
            fill_borders(yp)
            yp_tiles[(b, ct)] = yp

        def produce_yp_steady(b, ct, pre, act2=False):
            """norm + depthwise pipeline, steady-state variant.  act2: run
            TWO taps on ACT instead of one -- used for the ct=1 tile whose
            completion gates the big-conv ct1 pass ~15.5us into each batch
            (DVE is the tighter engine there; ACT has slack)."""
            wsp, wpt, bis = pre["wsp"], pre["wpt"], pre["bis"]
            xpb = xpb_pool.tile([128, FLAT], bf16, tag="xpb")
            xpb2 = xpb2_pool.tile([128, FLAT], bf16, tag="xpb2")
            nc.vector.memset(xpb[:, 0:MARG], 0.0)
            nc.vector.memset(xpb[:, FLAT - 2 : FLAT], 0.0)
            sumx = small_pool.tile([128, NQ], f32, tag="sumx")
            sumsq = small_pool.tile([128, 2], f32, tag="sumsq")
            convert_passes(pre, xpb, sumx, dve_borders=False,
                           sumsq=sumsq, xpb2=xpb2)
            fill_borders(xpb)

            # shifted copy (one element left) for 4B-aligned odd-offset
            # taps (DVE bf16 2x mode). NOTE: gpsimd bulk ops are poison here
            # -- they hold the shared DVE/GpSimd SBUF port for their whole
            # duration and stall every DVE tensor_tensor op; stays on ACT.
            shift_copy(xpb, xpb2)

            _, bconst, wsc = stats_finalize(sumx, sumsq, wsp, wpt, bis,
                                            scale_taps=True)

            yp = yp_pool.tile([128, FLAT], bf16, tag="yp")
            yp_seg = yp[:, OUT0 : OUT0 + OLEN]
            tap_src = tap_src_f(xpb, xpb2)

            # center tap's multiply runs on ACT (it has slack); the other
            # taps' products come from DVE 2x-mode tensor_scalar muls, then
            # are combined with a pairwise ADD TREE (same op count as a
            # serial chain but 4x shorter dependency depth and ~2x better
            # bf16 rounding error).  ACT muls are split in halves so they
            # never block the ACT queue for >2us.
            def act_mul(buf, tap):
                t = tap_idx(*tap)
                h = OLEN // 2
                nc.scalar.mul(buf[:, :h], tap_src(*tap)[:, :h],
                              wsc[:, t : t + 1])
                nc.scalar.mul(buf[:, h:], tap_src(*tap)[:, h:],
                              wsc[:, t : t + 1])

            tmpa = tmp_pool.tile([128, OLEN], bf16, tag="dwtmpa")
            act_mul(tmpa, (0, 0))

            # tap0 writes yp_seg = w0*x0 + B directly
            d0, w0 = ALL_TAPS[0]
            t0 = tap_idx(d0, w0)
            nc.vector.tensor_scalar(
                yp_seg, tap_src(d0, w0), wsc[:, t0 : t0 + 1], bconst[:],
                op0=ALU.mult, op1=ALU.add,
            )

            def mul_into(buf, tap):
                t = tap_idx(*tap)
                nc.vector.tensor_scalar_mul(buf[:], tap_src(*tap), wsc[:, t : t + 1])

            ta = tmp_pool.tile([128, OLEN], bf16, tag="dwA", bufs=1, name=f"dwA_{b}_{ct}")
            tb = tmp_pool.tile([128, OLEN], bf16, tag="dwB", bufs=1, name=f"dwB_{b}_{ct}")
            tc_ = tmp_pool.tile([128, OLEN], bf16, tag="dwC", bufs=1, name=f"dwC_{b}_{ct}")
            if act2:
                # second ACT tap (1,0): DVE drops to 6 muls + 8 adds
                tmpd = tmp_pool.tile([128, OLEN], bf16, tag="dwtmpd",
                                     bufs=1, name=f"dwtmpd_{b}_{ct}")
                act_mul(tmpd, (1, 0))
                rest = [t for t in ALL_TAPS[1:] if t not in ((0, 0), (1, 0))]
                mul_into(ta, rest[0])
                mul_into(tb, rest[1])
                nc.vector.tensor_add(ta[:], ta[:], tb[:])
                mul_into(tb, rest[2])
                mul_into(tc_, rest[3])
                nc.vector.tensor_add(tb[:], tb[:], tc_[:])
                nc.vector.tensor_add(ta[:], ta[:], tb[:])    # 4 DVE taps
                mul_into(tb, rest[4])
                mul_into(tc_, rest[5])
                nc.vector.tensor_add(tb[:], tb[:], tc_[:])
                nc.vector.tensor_add(tc_[:], tmpa[:], tmpd[:])  # ACT pair
                nc.vector.tensor_add(tb[:], tb[:], tc_[:])
                nc.vector.tensor_add(yp_seg, yp_seg, ta[:])
                nc.vector.tensor_add(yp_seg, yp_seg, tb[:])
            else:
                rest = [t for t in ALL_TAPS[1:] if t != (0, 0)]  # 7 taps
                mul_into(ta, rest[0])
                mul_into(tb, rest[1])
                nc.vector.tensor_add(ta[:], ta[:], tb[:])
                mul_into(tb, rest[2])
                mul_into(tc_, rest[3])
                nc.vector.tensor_add(tb[:], tb[:], tc_[:])
                nc.vector.tensor_add(ta[:], ta[:], tb[:])        # taps 1-4
                mul_into(tb, rest[4])
                mul_into(tc_, rest[5])
                nc.vector.tensor_add(tb[:], tb[:], tc_[:])
                mul_into(tc_, rest[6])
                nc.vector.tensor_add(tc_[:], tc_[:], tmpa[:])    # + ACT tap
                nc.vector.tensor_add(tb[:], tb[:], tc_[:])       # taps 5-7 + act
                nc.vector.tensor_add(yp_seg, yp_seg, ta[:])
                nc.vector.tensor_add(yp_seg, yp_seg, tb[:])

            nc.vector.memset(yp[:, 0:MARG], 0.0)
            nc.vector.memset(yp[:, FLAT - 2 : FLAT], 0.0)
            fill_borders(yp)
            yp_tiles[(b, ct)] = yp

        # tap order for the ct-outer paths: center tap first -- its rhs
        # reads only interior yp cells, so the bank-start matmuls don't
        # wait on the border-fill ACT ops.
        CT_OUTER_TAPS = [(0, 0)] + [
            (dh, dw) for dh in (-1, 0, 1) for dw in (-1, 0, 1) if (dh, dw) != (0, 0)
        ]

        def drain_bank(b, ot, stage, r0, nr, p, out_slice, dma_map=None):
            src = p[:].rearrange("p (r c) -> p r c", c=W)
            nc.scalar.activation(
                out=stage[:, r0 : r0 + nr, :], in_=src,
                func=AF.Identity, bias=cb_sb[:, ot : ot + 1],
            )
            rend = r0 + nr
            if dma_map is not None:
                seg = dma_map.get(rend)
                if seg is None:
                    return
                s0, rend = seg
            else:
                if rend % out_slice != 0:
                    return
                s0 = rend - out_slice
            nc.sync.dma_start(
                out=out_ext[b, ot * 128 : (ot + 1) * 128, s0:rend],
                in_=stage[:, s0:rend],
            )

        def big_conv(b):
            """Structure (per ot): for ot0 of batches>0, the ct0 taps run
            tap-outer first, giving PE ~15.5us of runway on yp[b,0] alone
            while DVE finishes yp[b,1].  Everything else runs BANK-OUTER
            with an immediate per-bank drain, so PSUM banks recycle
            continuously -- the old all-8-banks-finish-at-once shape made
            every ot/batch transition stall ~2-3us on queued ACT drains."""
            last = b == nb - 1
            for ot in range(OT):
                stage = stage_pool.tile([128, H, W], bf16, tag="stage")
                # The very last bank of the kernel tapers 8 -> 6+2 rows
                # (same matmul columns): the end-of-kernel barrier waits on
                # [last matmul -> drain -> out DMA -> HBM receipt], and a
                # 2-row drain + 2-row DMA shortens that chain ~1.5us.
                final = last and ot == OT - 1
                out_slice = 8 if final else OUT_SLICE
                if final:
                    blocks = ROW_BLOCKS[:7] + [(56, 6), (62, 2)]
                    dma_map = {8: (0, 8), 16: (8, 16), 24: (16, 24),
                               32: (24, 32), 40: (32, 40), 48: (40, 48),
                               56: (48, 56), 62: (56, 62), 64: (62, 64)}
                else:
                    blocks = ROW_BLOCKS
                    dma_map = None
                ps = {}
                for r0, nr in blocks:
                    ps[r0] = psum_pool.tile(
                        [128, nr * W], f32, tag="ps",
                        name=f"ps_{b}_{ot}_{r0}",
                    )
                runway = ot == 0 and b > 0
                if runway:
                    ypg = grid(yp_tiles[(b, 0)][:])
                    for ti, (dh, dw) in enumerate(CT_OUTER_TAPS):
                        lhsT = cw_sb[0][:, dh + 1, dw + 1, ot, :]
                        for r0, nr in ROW_BLOCKS:
                            rhs = ypg[:, r0 + 1 + dh : r0 + 1 + dh + nr,
                                      1 + dw : 1 + dw + W]
                            nc.tensor.matmul(ps[r0][:], lhsT, rhs,
                                             start=(ti == 0), stop=False)
                rest_cts = [1] if runway else list(range(CT))
                n_acc = len(rest_cts) * 9
                for r0, nr in blocks:
                    i = 0
                    for ct in rest_cts:
                        ypg = grid(yp_tiles[(b, ct)][:])
                        for dh, dw in CT_OUTER_TAPS:
                            lhsT = cw_sb[ct][:, dh + 1, dw + 1, ot, :]
                            rhs = ypg[:, r0 + 1 + dh : r0 + 1 + dh + nr,
                                      1 + dw : 1 + dw + W]
                            nc.tensor.matmul(
                                ps[r0][:], lhsT, rhs,
                                start=(not runway and i == 0),
                                stop=(i == n_acc - 1),
                            )
                            i += 1
                    drain_bank(b, ot, stage, r0, nr, ps[r0], out_slice,
                               dma_map)

        # Emission order doubles as DMA-issue order (Sync engine serializes
        # issues at ~0.65us each): batch 0's x slices go absolutely first,
        # then the weights, then each later batch's x prefetches interleave
        # ahead of the previous batch's big_conv.
        pe_warmup()
        pre = {}
        pre[(0, 0)] = prefetch_x(0, 0, wsp_early=True)
        pre[(0, 1)] = prefetch_x(0, 1)
        dw_finish = produce_yp_pe(0, 0, pre[(0, 0)])
        load_cw()
        produce_yp_early(0, 1, pre[(0, 1)], post_stats_hook=dw_finish)
        for b in range(nb):
            if b + 1 < nb:
                pre[(b + 1, 0)] = prefetch_x(b + 1, 0)
                pre[(b + 1, 1)] = prefetch_x(b + 1, 1)
            big_conv(b)
            if b + 1 < nb:
                produce_yp_steady(b + 1, 0, pre[(b + 1, 0)])
                produce_yp_steady(b + 1, 1, pre[(b + 1, 1)], act2=True)

    nc.compile()
    return nc


def _host_prep(x, w_spatial, w_pointwise, bias, conv_w, conv_b, nb=NB):
    import ml_dtypes

    ncores = x.shape[0] // nb
    cw = np.ascontiguousarray(
        conv_w.reshape(OT, 128, CT, 128, 3, 3).transpose(2, 3, 4, 5, 0, 1)
    ).astype(ml_dtypes.bfloat16)
    cb = np.ascontiguousarray(conv_b.reshape(OT, 128)).astype(np.float32)
    wsp = np.ascontiguousarray(w_spatial.reshape(-1, CT, 128, 9)).astype(np.float32)
    wpt = np.ascontiguousarray(w_pointwise.reshape(-1, CT, 128)).astype(np.float32)
    bis = np.ascontiguousarray(bias.reshape(-1, CT, 128)).astype(np.float32)
    x = np.ascontiguousarray(x).astype(ml_dtypes.bfloat16)
    in_maps = []
    for i in range(ncores):
        sl = slice(i * nb, (i + 1) * nb)
        in_maps.append({
            "x": np.ascontiguousarray(x[sl]),
            "wsp": np.ascontiguousarray(wsp[sl]),
            "wpt": np.ascontiguousarray(wpt[sl]),
            "bis": np.ascontiguousarray(bis[sl]),
            "cw": cw,
            "cb": cb,
        })
    return in_maps


def _run(inputs, trace=False):
    from concourse.bass_utils import run_bass_kernel_spmd

    if "nc" not in _CACHED:
        _CACHED["nc"] = _build()
    nc = _CACHED["nc"]
    in_maps = _host_prep(**inputs)
    kw = {}
    if trace:
        import shutil
        tdir = "/tmp/kernel_trace_out"
        shutil.rmtree(tdir, ignore_errors=True)
        os.makedirs(tdir, exist_ok=True)
        kw["tmpdir"] = tdir
    res = run_bass_kernel_spmd(
        nc, in_maps, core_ids=list(range(N_CORES)), trace=trace, **kw
    )
    out = np.concatenate([res.results[i]["out"] for i in range(N_CORES)], axis=0)
    return out.astype(np.float32), res


def kernel(x, w_spatial, w_pointwise, bias, conv_w, conv_b):
    out, _ = _run(
        dict(x=np.asarray(x), w_spatial=np.asarray(w_spatial),
             w_pointwise=np.asarray(w_pointwise), bias=np.asarray(bias),
             conv_w=np.asarray(conv_w), conv_b=np.asarray(conv_b)),
        trace=bool(int(os.environ.get("KERNEL_TRACE", "0"))),
    )
    return out

